# revision 1
# baseline (speedup 1.0000x reference)
import sys

sys.path.insert(0, "/opt/trn_rl_repo")

import numpy as np
import ml_dtypes

# Phi3SeerAttention, B=1 S=2048 HIDDEN=3072, H=32 q heads, HK=8 kv heads,
# D=96, gate block 64, gate hidden 128. Sharded TP over kv heads: core c
# owns kv head c and q heads 4c..4c+3; o-proj row-sharded, partials summed
# on host (the gather step).
H, HK, D, BLK, GH = 32, 8, 96, 64, 128
S, HIDDEN = 2048, 3072
G = H // HK          # 4 q heads per kv head (per core)
NB = S // BLK        # 32 gate blocks
KT = HIDDEN // 128   # 24 contraction tiles
NS = S // 512        # 4 sequence chunks of 512
NT = S // 128        # 16 t-tiles of 128
NE = HIDDEN // 512   # 6 output column chunks
NCORES = 8
THR = 0.03

_prog = None


def _build(debug=False):
    from concourse import bass, mybir, bacc
    import concourse.tile as tile
    from contextlib import ExitStack

    dt = mybir.dt
    BF, F32 = dt.bfloat16, dt.float32
    AF = mybir.ActivationFunctionType
    OP = mybir.AluOpType
    AX = mybir.AxisListType.X

    nc = bacc.Bacc()
    xt_d = nc.dram_tensor("xt", [HIDDEN, S], BF, kind="ExternalInput")
    wq_d = nc.dram_tensor("wq", [HIDDEN, G * D], BF, kind="ExternalInput")
    wk_d = nc.dram_tensor("wk", [HIDDEN, D], BF, kind="ExternalInput")
    wv_d = nc.dram_tensor("wv", [HIDDEN, D], BF, kind="ExternalInput")
    ow_d = nc.dram_tensor("ow", [G * D, HIDDEN], BF, kind="ExternalInput")
    cosq_d = nc.dram_tensor("cosq", [D, S], BF, kind="ExternalInput")
    sinq_d = nc.dram_tensor("sinq", [D, S], BF, kind="ExternalInput")
    cosk_d = nc.dram_tensor("cosk", [D, S], BF, kind="ExternalInput")
    sink_d = nc.dram_tensor("sink", [D, S], BF, kind="ExternalInput")
    rot_d = nc.dram_tensor("rot", [D, D], BF, kind="ExternalInput")
    gwq_d = nc.dram_tensor("gwq", [D, GH], F32, kind="ExternalInput")
    gwk_d = nc.dram_tensor("gwk", [2 * D, GH], F32, kind="ExternalInput")
    eye_d = nc.dram_tensor("eye32", [NB, NB], F32, kind="ExternalInput")
    emat_d = nc.dram_tensor("emat", [NB, NT * 128], F32, kind="ExternalInput")
    bcm_d = nc.dram_tensor("bcm", [NB, NB], F32, kind="ExternalInput")
    cmask_d = nc.dram_tensor("cmask", [128, 4 * 512], BF, kind="ExternalInput")
    out_d = nc.dram_tensor("out_p", [S, HIDDEN], BF, kind="ExternalOutput")

    # Raw (persistent) SBUF tensors that cross the phase-1 barrier. The two
    # TileContexts are separated by a full drain+barrier so no instruction
    # ever needs to wait on the union of all 8 DMA HW queue semaphores
    # (compute-engine instructions have a small embedded sync-wait cap).
    q_sb = nc.alloc_sbuf_tensor("q_sbuf", [D, G, S], BF)
    k_sb = nc.alloc_sbuf_tensor("k_sbuf", [D, S], BF)
    v_sb = nc.alloc_sbuf_tensor("v_sbuf", [128, NT, D + 1], BF)
    qp_sb = nc.alloc_sbuf_tensor("qp_sbuf", [D, G, NB], F32)
    km_sb = nc.alloc_sbuf_tensor("km_sbuf", [D, NB], F32)
    kx_sb = nc.alloc_sbuf_tensor("kx_sbuf", [D, NB], F32)

    # ---- context A / phase 1: QKV projection + gate pooling ----
    with tile.TileContext(nc) as tc:
        with tc.tile_pool(name="xw", bufs=1) as xw, tc.tile_pool(
            name="ps1", bufs=6, space="PSUM"
        ) as ps1:
            xt_sb = xw.tile([128, KT, S], BF)
            wq_sb = xw.tile([128, KT, G * D], BF)
            wk_sb = xw.tile([128, KT, D], BF)
            wv_sb = xw.tile([128, KT, D], BF)
            for kt in range(KT):
                r = slice(kt * 128, (kt + 1) * 128)
                nc.sync.dma_start(wq_sb[:, kt, :], wq_d[r, :])
                nc.sync.dma_start(wk_sb[:, kt, :], wk_d[r, :])
                nc.sync.dma_start(wv_sb[:, kt, :], wv_d[r, :])
            nc.vector.memset(v_sb[:, :, D : D + 1], 1.0)

            for j in range(NS):
                sl = slice(j * 512, (j + 1) * 512)
                for kt in range(KT):
                    r = slice(kt * 128, (kt + 1) * 128)
                    nc.sync.dma_start(xt_sb[:, kt, sl], xt_d[r, sl])

                for hh in range(G + 1):  # 0..3 = q heads, 4 = k
                    ps = ps1.tile([128, 512], F32)
                    pq = ps[:D, :]
                    for kt in range(KT):
                        lhsT = (
                            wq_sb[:, kt, hh * D : (hh + 1) * D]
                            if hh < G
                            else wk_sb[:, kt, :]
                        )
                        nc.tensor.matmul(
                            pq,
                            lhsT,
                            xt_sb[:, kt, sl],
                            start=(kt == 0),
                            stop=(kt == KT - 1),
                        )
                    pr = pq.rearrange("p (b w) -> p b w", w=BLK)
                    bs = slice(j * 8, (j + 1) * 8)
                    if hh < G:
                        # block SUM; 1/BLK folded into gate scale
                        nc.vector.tensor_reduce(
                            qp_sb[:, hh, bs], pr, axis=AX, op=OP.add
                        )
                        nc.scalar.copy(q_sb[:, hh, sl], pq)
                    else:
                        # block SUM; 1/BLK folded into gwk rows on host
                        nc.vector.tensor_reduce(km_sb[:, bs], pr, axis=AX, op=OP.add)
                        nc.vector.tensor_reduce(kx_sb[:, bs], pr, axis=AX, op=OP.max)
                        nc.scalar.copy(k_sb[:, sl], pq)

                for ti in range(4 * j, 4 * (j + 1)):
                    ps = ps1.tile([128, 512], F32)
                    pv = ps[:, :D]
                    for kt in range(KT):
                        nc.tensor.matmul(
                            pv,
                            xt_sb[:, kt, ti * 128 : (ti + 1) * 128],
                            wv_sb[:, kt, :],
                            start=(kt == 0),
                            stop=(kt == KT - 1),
                        )
                    nc.scalar.copy(v_sb[:, ti, :D], pv)

    # ---- context B: gate, RoPE, attention, o-projection ----
    with tile.TileContext(nc) as tc:
        with ExitStack() as ctx:
            perm = ctx.enter_context(tc.tile_pool(name="perm", bufs=1))
            mask_sb = perm.tile([128, NT, NB], BF)
            rot_sb = perm.tile([D, D], BF)
            gwq_sb = perm.tile([D, GH], F32)
            gwk_sb = perm.tile([D, 2, GH], F32)
            eye_sb = perm.tile([NB, NB], F32)
            bcm_sb = perm.tile([NB, NB], F32)
            ones_sb = perm.tile([1, 128], BF)
            attn_sb = perm.tile([D, G, S], BF)  # normalized attn output^T
            cosq_sb = perm.tile([D, S], BF)
            sinq_sb = perm.tile([D, S], BF)
            cosk_sb = perm.tile([D, S], BF)
            sink_sb = perm.tile([D, S], BF)
            emat_sb = perm.tile([NB, NT * 128], F32)
            cmask_sb = perm.tile([128, 4 * 512], BF)
            ow_sb = perm.tile([D, G, HIDDEN], BF)

            nc.sync.dma_start(rot_sb[:], rot_d[:])
            nc.sync.dma_start(gwq_sb[:], gwq_d[:])
            nc.sync.dma_start(gwk_sb[:, 0, :], gwk_d[0:D, :])
            nc.sync.dma_start(gwk_sb[:, 1, :], gwk_d[D : 2 * D, :])
            nc.sync.dma_start(eye_sb[:], eye_d[:])
            nc.sync.dma_start(bcm_sb[:], bcm_d[:])
            nc.sync.dma_start(cosq_sb[:], cosq_d[:])
            nc.sync.dma_start(sinq_sb[:], sinq_d[:])
            nc.sync.dma_start(cosk_sb[:], cosk_d[:])
            nc.sync.dma_start(sink_sb[:], sink_d[:])
            nc.sync.dma_start(emat_sb[:], emat_d[:])
            nc.sync.dma_start(cmask_sb[:], cmask_d[:])
            for hh in range(G):
                nc.sync.dma_start(ow_sb[:, hh, :], ow_d[hh * D : (hh + 1) * D, :])
            nc.vector.memset(ones_sb[:], 1.0)

            # ---- phase 2: block gate (fp32) ----
            with tc.tile_pool(name="gp", bufs=1) as gp, tc.tile_pool(
                name="gps", bufs=1, space="PSUM"
            ) as gps, tc.tile_pool(name="gpsm", bufs=2, space="PSUM") as gpsm:
                t0 = gp.tile([D, NB], F32)
                qps = gp.tile([D, NB], F32)
                nc.vector.tensor_add(t0[:], qp_sb[:, 0, :], qp_sb[:, 1, :])
                nc.vector.tensor_add(qps[:], qp_sb[:, 2, :], qp_sb[:, 3, :])
                nc.vector.tensor_add(qps[:], t0[:], qps[:])

                kg_ps = gps.tile([NB, GH], F32)
                nc.tensor.matmul(kg_ps, km_sb[:], gwk_sb[:, 0, :], start=True, stop=False)
                nc.tensor.matmul(kg_ps, kx_sb[:], gwk_sb[:, 1, :], start=False, stop=True)
                qg_ps = gps.tile([NB, GH], F32)
                nc.tensor.matmul(qg_ps, qps[:], gwq_sb[:], start=True, stop=True)
                qg_sb = gp.tile([NB, GH], F32)
                kg_sb = gp.tile([NB, GH], F32)
                # fold mean-over-heads (1/G), block mean (1/BLK), GH^-0.5
                nc.scalar.mul(qg_sb[:], qg_ps[:], (1.0 / (G * BLK)) * GH**-0.5)
                nc.scalar.copy(kg_sb[:], kg_ps[:])

                qgT_ps = gps.tile([GH, NB], F32)
                nc.tensor.matmul(qgT_ps, qg_sb[:], eye_sb[:], start=True, stop=True)
                kgT_ps = gps.tile([GH, NB], F32)
                nc.tensor.matmul(kgT_ps, kg_sb[:], eye_sb[:], start=True, stop=True)
                qgT_sb = gp.tile([GH, NB], F32)
                kgT_sb = gp.tile([GH, NB], F32)
                nc.scalar.copy(qgT_sb[:], qgT_ps[:])
                nc.scalar.copy(kgT_sb[:], kgT_ps[:])

                lg_ps = gps.tile([NB, NB], F32)
                nc.tensor.matmul(lg_ps, qgT_sb[:], kgT_sb[:], start=True, stop=True)
                lg_sb = gp.tile([NB, NB], F32)
                nc.scalar.copy(lg_sb[:], lg_ps[:])
                lm_sb = gp.tile([NB, NB], F32)
                nc.vector.tensor_add(lm_sb[:], lg_sb[:], bcm_sb[:])
                ge_sb = gp.tile([NB, NB], F32)
                gsum = gp.tile([NB, 1], F32)
                nc.scalar.activation(ge_sb[:], lm_sb[:], AF.Exp, accum_out=gsum[:])
                grc = gp.tile([NB, 1], F32)
                nc.vector.reciprocal(grc[:], gsum[:])
                prob_sb = gp.tile([NB, NB], F32)
                nc.scalar.activation(prob_sb[:], ge_sb[:], AF.Copy, scale=grc[:])
                m01 = gp.tile([NB, NB], F32)
                nc.vector.tensor_scalar(m01[:], prob_sb[:], THR, None, op0=OP.is_ge)
                nc.vector.tensor_tensor(m01[:], m01[:], eye_sb[:], op=OP.max)
                # transpose: expansion partitions index k blocks, m01 rows
                # index q blocks
                m01t_ps = gps.tile([NB, NB], F32)
                nc.tensor.matmul(m01t_ps, m01[:], eye_sb[:], start=True, stop=True)
                m01t = gp.tile([NB, NB], F32)
                nc.scalar.copy(m01t[:], m01t_ps[:])

                if debug:
                    for nm, t in [
                        ("dlg", lg_sb),
                        ("dqg", qg_sb),
                        ("dkg", kg_sb),
                        ("dprob", prob_sb),
                        ("dm01", m01),
                    ]:
                        dd = nc.dram_tensor(
                            nm, list(t[:].shape), t[:].dtype, kind="ExternalOutput"
                        )
                        nc.sync.dma_start(dd[:], t[:])

                for i in range(NT):
                    mp = gpsm.tile([128, NB], F32)
                    nc.tensor.matmul(
                        mp,
                        emat_sb[:, i * 128 : (i + 1) * 128],
                        m01t[:],
                        start=True,
                        stop=True,
                    )
                    nc.scalar.copy(mask_sb[:, i, :], mp[:])

            # ---- phase 3: RoPE in place on q^T / k^T ----
            with tc.tile_pool(name="rp", bufs=4) as rp, tc.tile_pool(
                name="rps", bufs=4, space="PSUM"
            ) as rps:
                for hh in range(G + 1):
                    src = q_sb[:, hh, :] if hh < G else k_sb[:]
                    cs = cosq_sb if hh < G else cosk_sb
                    sn = sinq_sb if hh < G else sink_sb
                    for j in range(NS):
                        sl = slice(j * 512, (j + 1) * 512)
                        rt = rps.tile([D, 512], F32)
                        nc.tensor.matmul(rt, rot_sb[:], src[:, sl], start=True, stop=True)
                        t1 = rp.tile([D, 512], BF)
                        nc.vector.tensor_mul(t1[:], src[:, sl], cs[:, sl])
                        t2 = rp.tile([D, 512], BF)
                        nc.vector.tensor_mul(t2[:], rt[:], sn[:, sl])
                        nc.vector.tensor_add(src[:, sl], t1[:], t2[:])

            # ---- phase 4: masked attention (transposed P layout) ----
            from concourse.bass import AP

            with tc.tile_pool(name="ap_", bufs=4) as ap_, tc.tile_pool(
                name="sm", bufs=4
            ) as sm, tc.tile_pool(name="sps", bufs=3, space="PSUM") as sps, tc.tile_pool(
                name="pvs", bufs=2, space="PSUM"
            ) as pvs, tc.tile_pool(name="rbs", bufs=2, space="PSUM") as rbs:
                for hh in range(G):
                    for j in range(NS):
                        ssl = slice(j * 512, (j + 1) * 512)
                        pv_ps = pvs.tile([D + 1, 512], F32)
                        ntile = 4 * (j + 1)
                        for ti in range(ntile):
                            s_ps = sps.tile([128, 512], F32)
                            nc.tensor.matmul(
                                s_ps,
                                k_sb[:, ti * 128 : (ti + 1) * 128],
                                q_sb[:, hh, ssl],
                                start=True,
                                stop=True,
                                skip_group_check=True,
                            )
                            p_sb = ap_.tile([128, 512], BF)
                            nc.scalar.activation(p_sb[:], s_ps[:], AF.Exp)
                            if ti >= 4 * j:
                                r = ti - 4 * j
                                nc.vector.tensor_mul(
                                    p_sb[:],
                                    p_sb[:],
                                    cmask_sb[:, r * 512 : (r + 1) * 512],
                                )
                            msl = mask_sb[:, ti, j * 8 : (j + 1) * 8]
                            mb = AP(
                                tensor=msl.tensor,
                                offset=msl.offset,
                                ap=list(msl.ap) + [[0, BLK]],
                            )
                            p3 = p_sb[:].rearrange("p (b w) -> p b w", w=BLK)
                            nc.vector.tensor_tensor(p3, p3, mb, op=OP.mult)
                            nc.tensor.matmul(
                                pv_ps,
                                v_sb[:, ti, :],
                                p_sb[:],
                                start=(ti == 0),
                                stop=(ti == ntile - 1),
                                skip_group_check=True,
                            )
                        sr = sm.tile([1, 512], F32)
                        nc.scalar.copy(sr[:], pv_ps[D : D + 1, :])
                        rc = sm.tile([1, 512], F32)
                        nc.vector.reciprocal(rc[:], sr[:])
                        rcb = sm.tile([1, 512], BF)
                        nc.vector.tensor_copy(rcb[:], rc[:])
                        rb_ps = rbs.tile([D, 512], F32)
                        nc.tensor.matmul(
                            rb_ps, ones_sb[:, :D], rcb[:], start=True, stop=True
                        )
                        # HW: DVE may read only ONE input from PSUM
                        rb_sb = sm.tile([D, 512], F32)
                        nc.scalar.copy(rb_sb[:], rb_ps[:])
                        nc.vector.tensor_mul(
                            attn_sb[:, hh, ssl], pv_ps[:D, :], rb_sb[:]
                        )

            # ---- phase 5: o-projection partial ----
            with tc.tile_pool(name="op_", bufs=4) as op_, tc.tile_pool(
                name="ops", bufs=4, space="PSUM"
            ) as ops:
                for si in range(NT):
                    tsl = slice(si * 128, (si + 1) * 128)
                    for ej in range(NE):
                        esl = slice(ej * 512, (ej + 1) * 512)
                        o_ps = ops.tile([128, 512], F32)
                        for hh in range(G):
                            nc.tensor.matmul(
                                o_ps,
                                attn_sb[:, hh, tsl],
                                ow_sb[:, hh, esl],
                                start=(hh == 0),
                                stop=(hh == G - 1),
                            )
                        o_sb = op_.tile([128, 512], BF)
                        nc.scalar.copy(o_sb[:], o_ps[:])
                        nc.sync.dma_start(out_d[tsl, esl], o_sb[:])

            if debug:
                for nm, t in [
                    ("dq", q_sb),
                    ("dk", k_sb),
                    ("dv", v_sb),
                    ("dmask", mask_sb),
                    ("dqp", qp_sb),
                    ("dkm", km_sb),
                    ("dkx", kx_sb),
                    ("dattn", attn_sb),
                ]:
                    dd = nc.dram_tensor(
                        nm, list(t[:].shape), t[:].dtype, kind="ExternalOutput"
                    )
                    nc.sync.dma_start(dd[:], t[:])
    return nc


def _host_prep(hidden_states, cos, sin, qkv_w, o_w, gate_wq, gate_wk):
    bf = ml_dtypes.bfloat16
    X = np.asarray(hidden_states, np.float32).reshape(S, HIDDEN)
    qkv_w = np.asarray(qkv_w, np.float32)
    o_w = np.asarray(o_w, np.float32)
    cos = np.asarray(cos, np.float32)
    sin = np.asarray(sin, np.float32)

    xt = np.ascontiguousarray(X.T).astype(bf)
    scale = D**-0.5
    cosT = np.ascontiguousarray(cos.T)
    sinT = np.ascontiguousarray(sin.T)
    cosq = (cosT * scale).astype(bf)
    sinq = (sinT * scale).astype(bf)
    cosk = cosT.astype(bf)
    sink = sinT.astype(bf)

    rt = np.zeros((D, D), np.float32)
    h = D // 2
    rt[np.arange(h) + h, np.arange(h)] = -1.0
    rt[np.arange(h), np.arange(h) + h] = 1.0
    rt = rt.astype(bf)

    emat = np.zeros((NB, NT * 128), np.float32)
    for i in range(NT):
        for p in range(128):
            emat[2 * i + p // BLK, i * 128 + p] = 1.0
    eye = np.eye(NB, dtype=np.float32)

    bcm = np.where(
        np.arange(NB)[None, :] <= np.arange(NB)[:, None], 0.0, -60.0
    ).astype(np.float32)
    # cmask[p, r*512+col] = 1 if col - p >= 128*r (k token ti*128+p causal
    # w.r.t. q token j*512+col on diagonal tiles, r = ti - 4j)
    p_i = np.arange(128)[:, None]
    cmask = np.zeros((128, 4 * 512), np.float32)
    for r in range(4):
        col = np.arange(512)[None, :]
        cmask[:, r * 512 : (r + 1) * 512] = (col - p_i >= 128 * r).astype(
            np.float32
        )
    cmask = cmask.astype(bf)

    # k block mean is computed on-device as a SUM; fold 1/BLK into the
    # mean-pool half of gate_wk
    gwk_s = np.asarray(gate_wk, np.float32).copy()
    gwk_s[:D, :] *= 1.0 / BLK

    common = dict(
        xt=xt,
        cosq=cosq,
        sinq=sinq,
        cosk=cosk,
        sink=sink,
        rot=rt,
        gwq=np.asarray(gate_wq, np.float32),
        gwk=gwk_s,
        eye32=eye,
        emat=emat,
        bcm=bcm,
        cmask=cmask,
    )
    maps = []
    for c in range(NCORES):
        maps.append(
            dict(
                common,
                wq=qkv_w[:, c * G * D : (c + 1) * G * D].astype(bf),
                wk=qkv_w[:, H * D + c * D : H * D + (c + 1) * D].astype(bf),
                wv=qkv_w[
                    :, H * D + HK * D + c * D : H * D + HK * D + (c + 1) * D
                ].astype(bf),
                ow=o_w[c * G * D : (c + 1) * G * D, :].astype(bf),
            )
        )
    return maps


def _gather(results):
    acc = np.zeros((S, HIDDEN), np.float32)
    for r in results:
        acc += np.asarray(r["out_p"]).astype(np.float32)
    return acc.reshape(1, S, HIDDEN)


def _run(inputs, trace=False):
    global _prog
    if _prog is None:
        _prog = _build()
        if not _prog.is_finalized():
            _prog.finalize()
    from concourse import bass_utils

    maps = _host_prep(**inputs)
    res = bass_utils.run_bass_kernel_spmd(
        _prog, maps, list(range(NCORES)), trace=trace
    )
    return _gather(res.results), res


def kernel(**inputs):
    out, _ = _run(inputs, trace=False)
    return out



# revision 5
# speedup vs baseline: 1.1820x; 1.1820x over previous
import sys

sys.path.insert(0, "/opt/trn_rl_repo")

import numpy as np
import ml_dtypes

# Phi3SeerAttention, B=1 S=2048 HIDDEN=3072, H=32 q heads, HK=8 kv heads,
# D=96, gate block 64, gate hidden 128. Sharded TP over kv heads: core c
# owns kv head c and q heads 4c..4c+3; o-proj row-sharded, partials summed
# on host (the gather step).
H, HK, D, BLK, GH = 32, 8, 96, 64, 128
S, HIDDEN = 2048, 3072
G = H // HK          # 4 q heads per kv head (per core)
NB = S // BLK        # 32 gate blocks
KT = HIDDEN // 128   # 24 contraction tiles
NS = S // 512        # 4 sequence chunks of 512
NT = S // 128        # 16 t-tiles of 128
NE = HIDDEN // 512   # 6 output column chunks
NR = G * D // 128    # 3 packed o-proj contraction tiles
NCORES = 8
THR = 0.03
SCALE = float(D) ** -0.5

_prog = None


def _build(debug=False):
    from concourse import bass, mybir, bacc, library_config
    import concourse.tile as tile
    from concourse.bass import AP

    dt = mybir.dt
    BF, F32 = dt.bfloat16, dt.float32
    AF = mybir.ActivationFunctionType
    OP = mybir.AluOpType
    AX = mybir.AxisListType.X

    nc = bacc.Bacc()
    xt_d = nc.dram_tensor("xt", [HIDDEN, S], BF, kind="ExternalInput")
    wq_d = nc.dram_tensor("wq", [HIDDEN, G * D], BF, kind="ExternalInput")
    wk_d = nc.dram_tensor("wk", [HIDDEN, D], BF, kind="ExternalInput")
    wv_d = nc.dram_tensor("wv", [HIDDEN, D], BF, kind="ExternalInput")
    owp_d = nc.dram_tensor("owp", [128, NR, HIDDEN], BF, kind="ExternalInput")
    cos_d = nc.dram_tensor("cosT", [D, S], BF, kind="ExternalInput")
    sin_d = nc.dram_tensor("sinT", [D, S], BF, kind="ExternalInput")
    rot_d = nc.dram_tensor("rot", [D, D], BF, kind="ExternalInput")
    gwq_d = nc.dram_tensor("gwq", [D, GH], F32, kind="ExternalInput")
    gwk_d = nc.dram_tensor("gwk", [2 * D, GH], F32, kind="ExternalInput")
    eye_d = nc.dram_tensor("eye32", [NB, NB], F32, kind="ExternalInput")
    emat_d = nc.dram_tensor("emat", [NB, NT * 128], F32, kind="ExternalInput")
    bcm_d = nc.dram_tensor("bcm", [NB, NB], F32, kind="ExternalInput")
    cm_d = nc.dram_tensor("cm128", [128, 128], BF, kind="ExternalInput")
    out_d = nc.dram_tensor("out_p", [S, HIDDEN], BF, kind="ExternalOutput")

    # Persistent SBUF tensors (live across the whole pipeline).
    q_sb = nc.alloc_sbuf_tensor("q_sbuf", [D, G, S], BF)
    k_sb = nc.alloc_sbuf_tensor("k_sbuf", [D, S], BF)
    v_sb = nc.alloc_sbuf_tensor("v_sbuf", [128, NT, D + 1], BF)
    qp_sb = nc.alloc_sbuf_tensor("qp_sbuf", [D, G, NB], F32)
    km_sb = nc.alloc_sbuf_tensor("km_sbuf", [D, NB], F32)
    kx_sb = nc.alloc_sbuf_tensor("kx_sbuf", [D, NB], F32)
    mask_sb = nc.alloc_sbuf_tensor("mask_sbuf", [128, NT, NB], BF)

    with tile.TileContext(nc) as tc:
        with tc.tile_pool(name="perm", bufs=1) as perm, tc.tile_pool(
            name="xw", bufs=2
        ) as xw, tc.tile_pool(name="gp", bufs=1) as gp, tc.tile_pool(
            name="ap_", bufs=4
        ) as ap_, tc.tile_pool(name="sm", bufs=4) as sm, tc.tile_pool(
            name="ar", bufs=2
        ) as ar, tc.tile_pool(name="apk", bufs=2) as apk, tc.tile_pool(
            name="ob", bufs=2
        ) as ob, tc.tile_pool(
            name="qps", bufs=2, space="PSUM"
        ) as qps, tc.tile_pool(name="sps", bufs=2, space="PSUM") as sps, tc.tile_pool(
            name="pvs", bufs=2, space="PSUM"
        ) as pvs, tc.tile_pool(name="ops", bufs=2, space="PSUM") as ops:
            # ---- permanent small tensors ----
            wq_sb = perm.tile([128, KT, G * D], BF)
            wk_sb = perm.tile([128, KT, D], BF)
            wv_sb = perm.tile([128, KT, D], BF)
            owp_sb = perm.tile([128, NR, HIDDEN], BF)
            cos_sb = perm.tile([D, S], BF)
            sin_sb = perm.tile([D, S], BF)
            rot_sb = perm.tile([D, D], BF)
            gwq_sb = perm.tile([D, GH], F32)
            gwk_sb = perm.tile([D, 2, GH], F32)
            eye_sb = perm.tile([NB, NB], F32)
            bcm_sb = perm.tile([NB, NB], F32)
            emat_sb = perm.tile([NB, NT * 128], F32)
            cm_sb = perm.tile([128, 128], BF)
            ones_sb = perm.tile([1, 128], BF)

            nc.sync.dma_start(wq_sb[:], wq_d[:].rearrange("(k p) n -> p k n", p=128))
            nc.sync.dma_start(wk_sb[:], wk_d[:].rearrange("(k p) n -> p k n", p=128))
            nc.sync.dma_start(wv_sb[:], wv_d[:].rearrange("(k p) n -> p k n", p=128))
            nc.sync.dma_start(owp_sb[:], owp_d[:])
            nc.sync.dma_start(cos_sb[:], cos_d[:])
            nc.sync.dma_start(sin_sb[:], sin_d[:])
            nc.sync.dma_start(rot_sb[:], rot_d[:])
            nc.sync.dma_start(gwq_sb[:], gwq_d[:])
            nc.sync.dma_start(gwk_sb[:, 0, :], gwk_d[0:D, :])
            nc.sync.dma_start(gwk_sb[:, 1, :], gwk_d[D : 2 * D, :])
            nc.sync.dma_start(eye_sb[:], eye_d[:])
            nc.sync.dma_start(bcm_sb[:], bcm_d[:])
            nc.sync.dma_start(emat_sb[:], emat_d[:])
            nc.sync.dma_start(cm_sb[:], cm_d[:])
            nc.vector.memset(ones_sb[:], 1.0)
            nc.vector.memset(v_sb[:, :, D : D + 1], 1.0)
            nc.gpsimd.load_library(library_config.standard)

            xt_src = xt_d[:].rearrange("(k p) s -> p k s", p=128)

            # ---- QKV projection, chunk-streamed (k first, then v, then q) ----
            for j in range(NS):
                sl = slice(j * 512, (j + 1) * 512)
                xt_sb = xw.tile([128, KT, 512], BF)
                nc.sync.dma_start(xt_sb[:], xt_src[:, :, sl])

                # k chunk
                ps = qps.tile([128, 512], F32)
                pk = ps[:D, :]
                for kt in range(KT):
                    nc.tensor.matmul(
                        pk,
                        wk_sb[:, kt, :],
                        xt_sb[:, kt, :],
                        start=(kt == 0),
                        stop=(kt == KT - 1),
                    )
                pr = pk.rearrange("p (b w) -> p b w", w=BLK)
                bs = slice(j * 8, (j + 1) * 8)
                # block SUM; 1/BLK folded into gwk rows on host
                nc.vector.tensor_reduce(km_sb[:, bs], pr, axis=AX, op=OP.add)
                nc.vector.tensor_reduce(kx_sb[:, bs], pr, axis=AX, op=OP.max)
                nc.scalar.copy(k_sb[:, sl], pk)

                # v tiles (transposed layout: s on partitions)
                for ti in range(4 * j, 4 * (j + 1)):
                    ps = qps.tile([128, 512], F32)
                    pv = ps[:, :D]
                    for kt in range(KT):
                        nc.tensor.matmul(
                            pv,
                            xt_sb[:, kt, (ti - 4 * j) * 128 : (ti - 4 * j + 1) * 128],
                            wv_sb[:, kt, :],
                            start=(kt == 0),
                            stop=(kt == KT - 1),
                        )
                    nc.scalar.copy(v_sb[:, ti, :D], pv)

                # q heads
                for hh in range(G):
                    ps = qps.tile([128, 512], F32)
                    pq = ps[:D, :]
                    for kt in range(KT):
                        nc.tensor.matmul(
                            pq,
                            wq_sb[:, kt, hh * D : (hh + 1) * D],
                            xt_sb[:, kt, :],
                            start=(kt == 0),
                            stop=(kt == KT - 1),
                        )
                    pr = pq.rearrange("p (b w) -> p b w", w=BLK)
                    # block SUM; 1/BLK folded into gate scale
                    nc.vector.tensor_reduce(qp_sb[:, hh, bs], pr, axis=AX, op=OP.add)
                    nc.scalar.copy(q_sb[:, hh, sl], pq)

                # RoPE in place for this chunk (k then q heads)
                for hh in range(G + 1):
                    src = k_sb[:, sl] if hh == G else q_sb[:, hh, sl]
                    rt = sps.tile([128, 512], F32, tag="s")
                    rr = rt[:D, :]
                    nc.tensor.matmul(rr, rot_sb[:], src, start=True, stop=True)
                    t1 = ap_.tile([D, 512], BF)
                    nc.gpsimd.tensor_tensor(t1[:], src, cos_sb[:, sl], op=OP.mult)
                    t2 = ap_.tile([D, 512], BF)
                    nc.vector.tensor_tensor(t2[:], rr, sin_sb[:, sl], op=OP.mult)
                    nc.vector.tensor_tensor(src, t1[:], t2[:], op=OP.add)

            # ---- gate: block-sparse mask (fp32, tiny) ----
            t0 = gp.tile([D, NB], F32)
            qsum = gp.tile([D, NB], F32)
            nc.vector.tensor_add(t0[:], qp_sb[:, 0, :], qp_sb[:, 1, :])
            nc.vector.tensor_add(qsum[:], qp_sb[:, 2, :], qp_sb[:, 3, :])
            nc.vector.tensor_add(qsum[:], t0[:], qsum[:])

            kg_ps = pvs.tile([NB, GH], F32, tag="pv")
            nc.tensor.matmul(kg_ps, km_sb[:], gwk_sb[:, 0, :], start=True, stop=False)
            nc.tensor.matmul(kg_ps, kx_sb[:], gwk_sb[:, 1, :], start=False, stop=True)
            qg_ps = pvs.tile([NB, GH], F32, tag="pv")
            nc.tensor.matmul(qg_ps, qsum[:], gwq_sb[:], start=True, stop=True)
            qg_sb = gp.tile([NB, GH], F32)
            kg_sb = gp.tile([NB, GH], F32)
            # fold mean-over-heads (1/G), block mean (1/BLK), GH^-0.5
            nc.scalar.mul(qg_sb[:], qg_ps[:], (1.0 / (G * BLK)) * GH**-0.5)
            nc.scalar.copy(kg_sb[:], kg_ps[:])

            qgT_ps = pvs.tile([GH, NB], F32, tag="pv")
            nc.tensor.matmul(qgT_ps, qg_sb[:], eye_sb[:], start=True, stop=True)
            kgT_ps = pvs.tile([GH, NB], F32, tag="pv")
            nc.tensor.matmul(kgT_ps, kg_sb[:], eye_sb[:], start=True, stop=True)
            qgT_sb = gp.tile([GH, NB], F32)
            kgT_sb = gp.tile([GH, NB], F32)
            nc.scalar.copy(qgT_sb[:], qgT_ps[:])
            nc.scalar.copy(kgT_sb[:], kgT_ps[:])

            lg_ps = pvs.tile([NB, NB], F32, tag="pv")
            nc.tensor.matmul(lg_ps, qgT_sb[:], kgT_sb[:], start=True, stop=True)
            lm_sb = gp.tile([NB, NB], F32)
            nc.vector.tensor_add(lm_sb[:], lg_ps[:], bcm_sb[:])
            ge_sb = gp.tile([NB, NB], F32)
            gsum = gp.tile([NB, 1], F32)
            nc.scalar.activation(ge_sb[:], lm_sb[:], AF.Exp, accum_out=gsum[:])
            grc = gp.tile([NB, 1], F32)
            nc.vector.reciprocal(grc[:], gsum[:])
            prob_sb = gp.tile([NB, NB], F32)
            nc.scalar.activation(prob_sb[:], ge_sb[:], AF.Copy, scale=grc[:])
            m01 = gp.tile([NB, NB], F32)
            nc.vector.tensor_scalar(m01[:], prob_sb[:], THR, None, op0=OP.is_ge)
            nc.vector.tensor_tensor(m01[:], m01[:], eye_sb[:], op=OP.max)
            # transpose so partitions index k blocks
            m01t_ps = pvs.tile([NB, NB], F32, tag="pv")
            nc.tensor.matmul(m01t_ps, m01[:], eye_sb[:], start=True, stop=True)
            m01t = gp.tile([NB, NB], F32)
            nc.scalar.copy(m01t[:], m01t_ps[:])

            if debug:
                for nm, t in [
                    ("dlg", lg_ps),
                    ("dqg", qg_sb),
                    ("dkg", kg_sb),
                    ("dprob", prob_sb),
                    ("dm01", m01),
                ]:
                    dd = nc.dram_tensor(
                        nm, list(t[:].shape), t[:].dtype, kind="ExternalOutput"
                    )
                    nc.sync.dma_start(dd[:], t[:])

            # expand k-block rows: mask_sb[p, i, :] = m01t[2i + p//64, :]
            for i in range(NT):
                mp = sps.tile([128, 512], F32, tag="s")
                mpn = mp[:, :NB]
                nc.tensor.matmul(
                    mpn,
                    emat_sb[:, i * 128 : (i + 1) * 128],
                    m01t[:],
                    start=True,
                    stop=True,
                )
                nc.scalar.copy(mask_sb[:, i, :], mpn)

            # ---- attention (j-outer, head-inner) + packed o-projection ----
            for j in range(NS):
                ssl = slice(j * 512, (j + 1) * 512)
                attn_raw = ar.tile([D, G, 512], BF)
                for hh in range(G):
                    pv_full = pvs.tile([128, 512], F32, tag="pv", name="pv_full")
                    pv_ps = pv_full[: D + 1, :]
                    ntile = 4 * (j + 1)
                    for ti in range(ntile):
                        r = ti - 4 * j
                        c0 = 128 * r if r > 0 else 0
                        cs = slice(c0, 512)
                        s_ps = sps.tile([128, 512], F32, tag="s")
                        nc.tensor.matmul(
                            s_ps[:, cs],
                            k_sb[:, ti * 128 : (ti + 1) * 128],
                            q_sb[:, hh, j * 512 + c0 : (j + 1) * 512],
                            start=True,
                            stop=True,
                            skip_group_check=True,
                        )
                        p_sb = ap_.tile([128, 512], BF)
                        nc.scalar.activation(p_sb[:, cs], s_ps[:, cs], AF.Exp, scale=SCALE)
                        if r >= 0:
                            # token-causal triangle on the leading 128 cols
                            nc.vector.tensor_tensor(
                                p_sb[:, c0 : c0 + 128],
                                p_sb[:, c0 : c0 + 128],
                                cm_sb[:],
                                op=OP.mult,
                            )
                        # gate block mask (broadcast over 64-col blocks) on Pool
                        b0 = j * 8 + (2 * r if r > 0 else 0)
                        msl = mask_sb[:, ti, b0 : (j + 1) * 8]
                        mb = AP(
                            tensor=msl.tensor,
                            offset=msl.offset,
                            ap=list(msl.ap) + [[0, BLK]],
                        )
                        p3 = p_sb[:, cs].rearrange("p (b w) -> p b w", w=BLK)
                        nc.gpsimd.tensor_tensor(p3, p3, mb, op=OP.mult)
                        nc.tensor.matmul(
                            pv_ps[:, cs],
                            v_sb[:, ti, :],
                            p_sb[:, cs],
                            start=(ti == 0),
                            stop=(ti == ntile - 1),
                            skip_group_check=True,
                        )
                    rcb = sm.tile([1, 512], BF)
                    with nc.allow_low_precision(reason="recip to bf16 as baseline"):
                        nc.vector.reciprocal(rcb[:], pv_ps[D : D + 1, :])
                    rb_ps = sps.tile([128, 512], F32, tag="s")
                    nc.tensor.matmul(
                        rb_ps[:D, :], ones_sb[:, :D], rcb[:], start=True, stop=True
                    )
                    rb_sb = sm.tile([D, 512], BF)
                    nc.scalar.copy(rb_sb[:], rb_ps[:D, :])
                    # DVE: one PSUM input max
                    nc.vector.tensor_tensor(
                        attn_raw[:, hh, :], pv_ps[:D, :], rb_sb[:], op=OP.mult
                    )

                # repack [96,4,512] -> [128,3,512] (partition-shift DMAs)
                attnp = apk.tile([128, NR, 512], BF)
                nc.sync.dma_start(attnp[0:96, 0, :], attn_raw[:, 0, :])
                nc.sync.dma_start(attnp[96:128, 0, :], attn_raw[0:32, 1, :])
                nc.sync.dma_start(attnp[0:64, 1, :], attn_raw[32:96, 1, :])
                nc.sync.dma_start(attnp[64:128, 1, :], attn_raw[0:64, 2, :])
                nc.sync.dma_start(attnp[0:32, 2, :], attn_raw[64:96, 2, :])
                nc.sync.dma_start(attnp[32:128, 2, :], attn_raw[:, 3, :])

                # packed o-projection for the 4 s-tiles of this chunk
                for si in range(4 * j, 4 * (j + 1)):
                    cc = (si - 4 * j) * 128
                    o_sb = ob.tile([128, HIDDEN], BF)
                    for ej in range(NE):
                        esl = slice(ej * 512, (ej + 1) * 512)
                        o_ps = ops.tile([128, 512], F32)
                        for rr_ in range(NR):
                            nc.tensor.matmul(
                                o_ps,
                                attnp[:, rr_, cc : cc + 128],
                                owp_sb[:, rr_, esl],
                                start=(rr_ == 0),
                                stop=(rr_ == NR - 1),
                            )
                        if ej % 2 == 0:
                            nc.scalar.copy(o_sb[:, esl], o_ps[:])
                        else:
                            nc.vector.tensor_copy(o_sb[:, esl], o_ps[:])
                    nc.sync.dma_start(
                        out_d[si * 128 : (si + 1) * 128, :], o_sb[:]
                    )

            if debug:
                for nm, t in [
                    ("dq", q_sb),
                    ("dk", k_sb),
                    ("dv", v_sb),
                    ("dmask", mask_sb),
                    ("dqp", qp_sb),
                    ("dkm", km_sb),
                    ("dkx", kx_sb),
                ]:
                    dd = nc.dram_tensor(
                        nm, list(t[:].shape), t[:].dtype, kind="ExternalOutput"
                    )
                    nc.sync.dma_start(dd[:], t[:])
    return nc


def _host_prep(hidden_states, cos, sin, qkv_w, o_w, gate_wq, gate_wk):
    bf = ml_dtypes.bfloat16
    X = np.asarray(hidden_states, np.float32).reshape(S, HIDDEN)
    qkv_w = np.asarray(qkv_w, np.float32)
    o_w = np.asarray(o_w, np.float32)
    cos = np.asarray(cos, np.float32)
    sin = np.asarray(sin, np.float32)

    xt = np.ascontiguousarray(X.T).astype(bf)
    cosT = np.ascontiguousarray(cos.T).astype(bf)
    sinT = np.ascontiguousarray(sin.T).astype(bf)

    rt = np.zeros((D, D), np.float32)
    h = D // 2
    rt[np.arange(h) + h, np.arange(h)] = -1.0
    rt[np.arange(h), np.arange(h) + h] = 1.0
    rt = rt.astype(bf)

    emat = np.zeros((NB, NT * 128), np.float32)
    for i in range(NT):
        for p in range(128):
            emat[2 * i + p // BLK, i * 128 + p] = 1.0
    eye = np.eye(NB, dtype=np.float32)

    bcm = np.where(
        np.arange(NB)[None, :] <= np.arange(NB)[:, None], 0.0, -60.0
    ).astype(np.float32)
    # cm128[p, c] = 1 if c >= p (token causal within a diagonal 128-block)
    p_i = np.arange(128)[:, None]
    c_i = np.arange(128)[None, :]
    cm = (c_i >= p_i).astype(np.float32).astype(bf)

    # k block mean is computed on-device as a SUM; fold 1/BLK into the
    # mean-pool half of gate_wk
    gwk_s = np.asarray(gate_wk, np.float32).copy()
    gwk_s[:D, :] *= 1.0 / BLK

    common = dict(
        xt=xt,
        cosT=cosT,
        sinT=sinT,
        rot=rt,
        gwq=np.asarray(gate_wq, np.float32),
        gwk=gwk_s,
        eye32=eye,
        emat=emat,
        bcm=bcm,
        cm128=cm,
    )
    maps = []
    for c in range(NCORES):
        ow_c = o_w[c * G * D : (c + 1) * G * D, :]  # [384, HIDDEN]
        owp = np.ascontiguousarray(
            ow_c.reshape(NR, 128, HIDDEN).transpose(1, 0, 2)
        ).astype(bf)
        maps.append(
            dict(
                common,
                wq=qkv_w[:, c * G * D : (c + 1) * G * D].astype(bf),
                wk=qkv_w[:, H * D + c * D : H * D + (c + 1) * D].astype(bf),
                wv=qkv_w[
                    :, H * D + HK * D + c * D : H * D + HK * D + (c + 1) * D
                ].astype(bf),
                owp=owp,
            )
        )
    return maps


def _gather(results):
    acc = np.zeros((S, HIDDEN), np.float32)
    for r in results:
        acc += np.asarray(r["out_p"]).astype(np.float32)
    return acc.reshape(1, S, HIDDEN)


def _run(inputs, trace=False):
    global _prog
    if _prog is None:
        _prog = _build()
        if not _prog.is_finalized():
            _prog.finalize()
    from concourse import bass_utils

    maps = _host_prep(**inputs)
    res = bass_utils.run_bass_kernel_spmd(
        _prog, maps, list(range(NCORES)), trace=trace
    )
    return _gather(res.results), res


def kernel(**inputs):
    out, _ = _run(inputs, trace=False)
    return out


# revision 34
# speedup vs baseline: 1.4944x; 1.2643x over previous
import sys

sys.path.insert(0, "/opt/trn_rl_repo")

import numpy as np
import ml_dtypes

# Phi3SeerAttention, B=1 S=2048 HIDDEN=3072, H=32 q heads, HK=8 kv heads,
# D=96, gate block 64, gate hidden 128. Sharded TP over kv heads: core c
# owns kv head c and q heads 4c..4c+3; o-proj row-sharded, partials summed
# on host (the gather step).
H, HK, D, BLK, GH = 32, 8, 96, 64, 128
S, HIDDEN = 2048, 3072
G = H // HK          # 4 q heads per kv head (per core)
NB = S // BLK        # 32 gate blocks
KT = HIDDEN // 128   # 24 contraction tiles
NS = S // 512        # 4 sequence chunks of 512
NT = S // 128        # 16 t-tiles of 128
NE = HIDDEN // 512   # 6 output column chunks
NR = G * D // 128    # 3 packed o-proj contraction tiles
NCORES = 8
THR = 0.03
SCALE = float(D) ** -0.5
GSCALE = (1.0 / (G * BLK)) * float(GH) ** -0.5

_prog = None


def _build(debug=False):
    from concourse import bass, mybir, bacc, library_config
    import concourse.tile as tile
    from concourse.bass import AP

    dt = mybir.dt
    BF, F32 = dt.bfloat16, dt.float32
    AF = mybir.ActivationFunctionType
    OP = mybir.AluOpType
    AX = mybir.AxisListType.X

    nc = bacc.Bacc()
    xt_d = nc.dram_tensor("xt", [HIDDEN, S], BF, kind="ExternalInput")
    wq_d = nc.dram_tensor("wq", [HIDDEN, G * D], BF, kind="ExternalInput")
    wk_d = nc.dram_tensor("wk", [HIDDEN, D], BF, kind="ExternalInput")
    wv_d = nc.dram_tensor("wv", [HIDDEN, D], BF, kind="ExternalInput")
    owp_d = nc.dram_tensor("owp", [128, NR, HIDDEN], BF, kind="ExternalInput")
    cos_d = nc.dram_tensor("cosT", [D, S], BF, kind="ExternalInput")
    sin_d = nc.dram_tensor("sinT", [D, S], BF, kind="ExternalInput")
    rot_d = nc.dram_tensor("rot", [D, D], BF, kind="ExternalInput")
    gwq_d = nc.dram_tensor("gwq", [D, GH], F32, kind="ExternalInput")
    gwk_d = nc.dram_tensor("gwk", [2 * D, GH], F32, kind="ExternalInput")
    eye_d = nc.dram_tensor("eye32", [NB, NB], F32, kind="ExternalInput")
    emat_d = nc.dram_tensor("emat", [NB, NT * 128], BF, kind="ExternalInput")
    bcm4_d = nc.dram_tensor("bcm4", [8, NS, NB], F32, kind="ExternalInput")
    eye4_d = nc.dram_tensor("eye4", [8, NS, NB], F32, kind="ExternalInput")
    cm_d = nc.dram_tensor("cm128", [128, 128], BF, kind="ExternalInput")
    out_d = nc.dram_tensor("out_p", [S, HIDDEN], BF, kind="ExternalOutput")

    # Persistent SBUF tensors (live across the whole pipeline).
    q_sb = nc.alloc_sbuf_tensor("q_sbuf", [D, G, S], BF)
    k_sb = nc.alloc_sbuf_tensor("k_sbuf", [D, S], BF)
    v_sb = nc.alloc_sbuf_tensor("v_sbuf", [128, NT, D + 1], BF)
    qp_sb = nc.alloc_sbuf_tensor("qp_sbuf", [D, G, NB], F32)
    km_sb = nc.alloc_sbuf_tensor("km_sbuf", [D, NB], F32)
    kx_sb = nc.alloc_sbuf_tensor("kx_sbuf", [D, NB], F32)
    mask_sb = nc.alloc_sbuf_tensor("mask_sbuf", [128, NT, NB], BF)

    with tile.TileContext(nc) as tc:
        with tc.tile_pool(name="perm", bufs=1) as perm, tc.tile_pool(
            name="xw", bufs=3
        ) as xw, tc.tile_pool(name="gp", bufs=1) as gp, tc.tile_pool(
            name="ap_", bufs=4
        ) as ap_, tc.tile_pool(name="sm", bufs=4) as sm, tc.tile_pool(
            name="ar", bufs=2
        ) as ar, tc.tile_pool(name="apk", bufs=2) as apk, tc.tile_pool(
            name="ob", bufs=2
        ) as ob, tc.tile_pool(
            name="qps", bufs=2, space="PSUM"
        ) as qps, tc.tile_pool(name="sps", bufs=2, space="PSUM") as sps, tc.tile_pool(
            name="pvs", bufs=2, space="PSUM"
        ) as pvs, tc.tile_pool(name="ops", bufs=2, space="PSUM") as ops:
            # ---- permanent small tensors ----
            wq_sb = perm.tile([128, KT, G * D], BF)
            wk_sb = perm.tile([128, KT, D], BF)
            wv_sb = perm.tile([128, KT, D], BF)
            owp_sb = perm.tile([128, NR, HIDDEN], BF)
            cos_sb = perm.tile([D, S], BF)
            sin_sb = perm.tile([D, S], BF)
            rot_sb = perm.tile([D, D], BF)
            gwq_sb = perm.tile([D, GH], F32)
            gwk_sb = perm.tile([D, 2, GH], F32)
            eye_sb = perm.tile([NB, NB], F32)
            bcm4_sb = perm.tile([8, NS, NB], F32)
            eye4_sb = perm.tile([8, NS, NB], F32)
            emat_sb = perm.tile([NB, NT * 128], BF)
            cm_sb = perm.tile([128, 128], BF)
            ones_sb = perm.tile([1, 128], BF)
            kgT_sb = perm.tile([GH, NB], F32)
            m01t_bf = perm.tile([NB, NB], BF)

            nc.vector.memset(ones_sb[:], 1.0)
            nc.vector.memset(v_sb[:, :, D : D + 1], 1.0)
            nc.vector.memset(kgT_sb[:], 0.0)
            nc.gpsimd.load_library(library_config.standard)

            xt_src = xt_d[:].rearrange("(k p) s -> p k s", p=128)

            # ---- input DMAs, latency-ordered for the serialized DMA device:
            # (wk, xt0) first so PE starts early; cold tables later.
            xt_tiles = []
            for j in range(NS):
                xt_tiles.append(xw.tile([128, KT, 512], BF, name="xt_sb", tag="xt"))
            wk_src = wk_d[:].rearrange("(k p) n -> p k n", p=128)
            nc.sync.dma_start(wk_sb[:, 0:3, :], wk_src[:, 0:3, :])
            nc.sync.dma_start(xt_tiles[0][:, 0:3, :], xt_src[:, 0:3, 0:512])
            nc.sync.dma_start(wk_sb[:, 3:24, :], wk_src[:, 3:24, :])
            nc.sync.dma_start(xt_tiles[0][:, 3:24, :], xt_src[:, 3:24, 0:512])
            nc.sync.dma_start(wv_sb[:], wv_d[:].rearrange("(k p) n -> p k n", p=128))
            nc.sync.dma_start(wq_sb[:], wq_d[:].rearrange("(k p) n -> p k n", p=128))
            nc.sync.dma_start(cos_sb[:], cos_d[:])
            nc.sync.dma_start(sin_sb[:], sin_d[:])
            nc.sync.dma_start(rot_sb[:], rot_d[:])
            nc.sync.dma_start(gwq_sb[:], gwq_d[:])
            nc.sync.dma_start(gwk_sb[:, 0, :], gwk_d[0:D, :])
            nc.sync.dma_start(gwk_sb[:, 1, :], gwk_d[D : 2 * D, :])
            nc.sync.dma_start(eye_sb[:], eye_d[:])
            nc.sync.dma_start(bcm4_sb[:], bcm4_d[:])
            nc.sync.dma_start(eye4_sb[:], eye4_d[:])
            nc.sync.dma_start(emat_sb[:], emat_d[:])
            nc.sync.dma_start(cm_sb[:], cm_d[:])
            nc.sync.dma_start(xt_tiles[1][:], xt_src[:, :, 512:1024])
            nc.sync.dma_start(owp_sb[:], owp_d[:])
            nc.sync.dma_start(xt_tiles[2][:], xt_src[:, :, 1024:1536])
            nc.sync.dma_start(xt_tiles[3][:], xt_src[:, :, 1536:2048])

            # Deferred o-projection: the previous chunk's (si, ej) groups are
            # emitted one per attention tile-slot of the NEXT chunk, so they
            # fill PE while attention is exp(ACT)-paced, and the PE stream
            # never blocks on a whole chunk's attention completing.
            ost = {"queue": [], "attnp": None, "osb": None, "j": 0, "n": 0}

            def emit_oproj_group(tail=False):
                if not ost["queue"]:
                    return False
                si, ej = ost["queue"].pop(0)
                jp = ost["j"]
                if ej == 0:
                    ost["osb"] = ob.tile([128, HIDDEN], BF, name="o_sb")
                o_sb = ost["osb"]
                cc = (si - 4 * jp) * 128
                esl = slice(ej * 512, (ej + 1) * 512)
                if tail:
                    # attention is over: rotate score/projection banks too
                    ost["n"] += 1
                    pool, tg = [(ops, ""), (sps, "s"), (qps, "ps")][ost["n"] % 3]
                    o_ps = pool.tile([128, 512], F32, name="o_ps", tag=tg)
                else:
                    o_ps = ops.tile([128, 512], F32, name="o_ps")
                for rr_ in range(NR):
                    nc.tensor.matmul(
                        o_ps,
                        ost["attnp"][:, rr_, cc : cc + 128],
                        owp_sb[:, rr_, esl],
                        start=(rr_ == 0),
                        stop=(rr_ == NR - 1),
                    )
                if tail and ej % 2 == 0:
                    nc.scalar.copy(o_sb[:, esl], o_ps[:])
                else:
                    nc.vector.tensor_copy(o_sb[:, esl], o_ps[:])
                if ej == NE - 1:
                    nc.sync.dma_start(out_d[si * 128 : (si + 1) * 128, :], o_sb[:])
                return True

            # ---- fused per-chunk pipeline: projections -> RoPE -> gate ->
            # attention -> deferred o-projection ----
            for j in range(NS):
                sl = slice(j * 512, (j + 1) * 512)
                xt_sb = xt_tiles[j]

                # k chunk
                ps = qps.tile([128, 512], F32)
                pk = ps[:D, :]
                for kt in range(KT):
                    nc.tensor.matmul(
                        pk,
                        wk_sb[:, kt, :],
                        xt_sb[:, kt, :],
                        start=(kt == 0),
                        stop=(kt == KT - 1),
                    )
                pr = pk.rearrange("p (b w) -> p b w", w=BLK)
                bs = slice(j * 8, (j + 1) * 8)
                # block SUM; 1/BLK folded into gwk rows on host
                nc.vector.tensor_reduce(km_sb[:, bs], pr, axis=AX, op=OP.add)
                nc.vector.tensor_reduce(kx_sb[:, bs], pr, axis=AX, op=OP.max)
                nc.scalar.copy(k_sb[:, sl], pk)

                # q heads (before v: the gate chain needs all q pools)
                for hh in range(G):
                    ps = qps.tile([128, 512], F32)
                    pq = ps[:D, :]
                    for kt in range(KT):
                        nc.tensor.matmul(
                            pq,
                            wq_sb[:, kt, hh * D : (hh + 1) * D],
                            xt_sb[:, kt, :],
                            start=(kt == 0),
                            stop=(kt == KT - 1),
                        )
                    pr = pq.rearrange("p (b w) -> p b w", w=BLK)
                    # block SUM; 1/BLK folded into gate scale
                    nc.vector.tensor_reduce(qp_sb[:, hh, bs], pr, axis=AX, op=OP.add)
                    nc.scalar.copy(q_sb[:, hh, sl], pq)

                # RoPE in place for this chunk (k then q heads)
                for hh in range(G + 1):
                    src = k_sb[:, sl] if hh == G else q_sb[:, hh, sl]
                    rt = sps.tile([128, 512], F32, tag="s")
                    rr = rt[:D, :]
                    nc.tensor.matmul(rr, rot_sb[:], src, start=True, stop=True)
                    t1 = ap_.tile([D, 512], BF)
                    nc.gpsimd.tensor_tensor(t1[:], src, cos_sb[:, sl], op=OP.mult)
                    t2 = ap_.tile([D, 512], BF)
                    nc.vector.tensor_tensor(t2[:], rr, sin_sb[:, sl], op=OP.mult)
                    nc.gpsimd.tensor_tensor(src, t1[:], t2[:], op=OP.add)

                # v tiles (transposed layout: s on partitions); PE filler
                # while the serial gate chain runs
                for ti in range(4 * j, 4 * (j + 1)):
                    ps = qps.tile([128, 512], F32)
                    pv = ps[:, :D]
                    for kt in range(KT):
                        nc.tensor.matmul(
                            pv,
                            xt_sb[:, kt, (ti - 4 * j) * 128 : (ti - 4 * j + 1) * 128],
                            wv_sb[:, kt, :],
                            start=(kt == 0),
                            stop=(kt == KT - 1),
                        )
                    nc.scalar.copy(v_sb[:, ti, :D], pv)

                # ---- incremental gate for this chunk's 8 q-blocks ----
                # softmax is row-wise over k-blocks <= q-block, so rows for
                # chunk j only need pools from chunks <= j. Projections are
                # emitted directly transposed (lhsT = gate weight), and the
                # gate scale (1/(G*BLK))*GH^-0.5 is folded into the gate exp
                # (bcm4 is pre-divided by it on host).
                t0g = gp.tile([D, 8], F32)
                qsum = gp.tile([D, 8], F32)
                nc.vector.tensor_add(t0g[:], qp_sb[:, 0, bs], qp_sb[:, 1, bs])
                nc.vector.tensor_add(qsum[:], qp_sb[:, 2, bs], qp_sb[:, 3, bs])
                nc.vector.tensor_add(qsum[:], t0g[:], qsum[:])

                eye8 = eye_sb[0:8, 0:8]
                kgT_ps = pvs.tile([GH, 8], F32, tag="pv", name="kgT_ps")
                nc.tensor.matmul(
                    kgT_ps, gwk_sb[:, 0, :], km_sb[:, bs], start=True, stop=False
                )
                nc.tensor.matmul(
                    kgT_ps, gwk_sb[:, 1, :], kx_sb[:, bs], start=False, stop=True
                )
                qgT_ps = pvs.tile([GH, 8], F32, tag="pv", name="qgT_ps")
                nc.tensor.matmul(qgT_ps, gwq_sb[:], qsum[:], start=True, stop=True)
                qgT_j = gp.tile([GH, 8], F32)
                nc.scalar.copy(qgT_j[:], qgT_ps[:])
                nc.scalar.copy(kgT_sb[:, bs], kgT_ps[:])

                lg_ps = pvs.tile([8, NB], F32, tag="pv", name="lg_ps")
                nc.tensor.matmul(lg_ps, qgT_j[:], kgT_sb[:], start=True, stop=True)
                lm_sb = gp.tile([8, NB], F32)
                nc.vector.tensor_add(lm_sb[:], lg_ps[:], bcm4_sb[:, j, :])
                ge_sb = gp.tile([8, NB], F32)
                gsum = gp.tile([8, 1], F32)
                nc.scalar.activation(
                    ge_sb[:], lm_sb[:], AF.Exp, scale=GSCALE, accum_out=gsum[:]
                )
                grc = gp.tile([8, 1], F32)
                nc.vector.reciprocal(grc[:], gsum[:])
                prob_sb = gp.tile([8, NB], F32)
                nc.scalar.activation(prob_sb[:], ge_sb[:], AF.Copy, scale=grc[:])
                m01 = gp.tile([8, NB], F32)
                nc.vector.tensor_scalar(m01[:], prob_sb[:], THR, None, op0=OP.is_ge)
                nc.vector.tensor_tensor(m01[:], m01[:], eye4_sb[:, j, :], op=OP.max)
                # transpose the 8 new rows into m01t columns
                m01tj_ps = pvs.tile([NB, 8], F32, tag="pv", name="m01tj_ps")
                nc.tensor.matmul(m01tj_ps, m01[:], eye8, start=True, stop=True)
                nc.scalar.copy(m01t_bf[:, bs], m01tj_ps[:])

                # expand k-block rows: mask_sb[p, ti, bs] = m01t[2ti+p//64, bs]
                for ti in range(4 * (j + 1)):
                    mp = sps.tile([128, 512], F32, tag="s", name="mp")
                    mpn = mp[:, :8]
                    nc.tensor.matmul(
                        mpn,
                        emat_sb[:, ti * 128 : (ti + 1) * 128],
                        m01t_bf[:, bs],
                        start=True,
                        stop=True,
                    )
                    nc.scalar.copy(mask_sb[:, ti, bs], mpn)

                # ---- attention for this chunk ----
                # The normalization tail of head h is emitted inside head
                # h+1's tile loop so the (mostly in-order) PE stream never
                # waits on the DVE reciprocal chain at a head boundary.
                attn_raw = ar.tile([D, G, 512], BF)
                attnp = apk.tile([128, NR, 512], BF)

                def norm_tail(hh, pv_ps, rcb):
                    rb_ps = sps.tile([128, 512], F32, tag="s", name="rb_ps")
                    nc.tensor.matmul(
                        rb_ps[:D, :], ones_sb[:, :D], rcb[:], start=True, stop=True
                    )
                    rb_sb = sm.tile([D, 512], BF, name="rb_sb")
                    nc.vector.tensor_copy(rb_sb[:], rb_ps[:D, :])
                    # DVE: one PSUM input max
                    nc.vector.tensor_tensor(
                        attn_raw[:, hh, :], pv_ps[:D, :], rb_sb[:], op=OP.mult
                    )
                    # repack [96,4,512] -> [128,3,512] as each head lands
                    if hh == 0:
                        nc.sync.dma_start(attnp[0:96, 0, :], attn_raw[:, 0, :])
                    elif hh == 1:
                        nc.sync.dma_start(attnp[96:128, 0, :], attn_raw[0:32, 1, :])
                        nc.sync.dma_start(attnp[0:64, 1, :], attn_raw[32:96, 1, :])
                    elif hh == 2:
                        nc.sync.dma_start(attnp[64:128, 1, :], attn_raw[0:64, 2, :])
                        nc.sync.dma_start(attnp[0:32, 2, :], attn_raw[64:96, 2, :])
                    else:
                        nc.sync.dma_start(attnp[32:128, 2, :], attn_raw[:, 3, :])

                pending = None
                slot = 0
                nslots = G * 4 * (j + 1)
                for hh in range(G):
                    pv_full = pvs.tile([128, 512], F32, tag="pv", name="pv_full")
                    pv_ps = pv_full[: D + 1, :]
                    ntile = 4 * (j + 1)
                    for ti in range(ntile):
                        if ti == 2 and pending is not None:
                            norm_tail(*pending)
                            pending = None
                        r = ti - 4 * j
                        c0 = 128 * r if r > 0 else 0
                        cs = slice(c0, 512)
                        # final chunk: projections are done, so alternate the
                        # score tiles into the idle qps banks (pipeline depth 4)
                        if j == NS - 1 and ti % 2 == 1:
                            s_ps = qps.tile([128, 512], F32, tag="ps", name="s_ps")
                        else:
                            s_ps = sps.tile([128, 512], F32, tag="s")
                        nc.tensor.matmul(
                            s_ps[:, cs],
                            k_sb[:, ti * 128 : (ti + 1) * 128],
                            q_sb[:, hh, j * 512 + c0 : (j + 1) * 512],
                            start=True,
                            stop=True,
                            skip_group_check=True,
                        )
                        p_sb = ap_.tile([128, 512], BF)
                        nc.scalar.activation(p_sb[:, cs], s_ps[:, cs], AF.Exp, scale=SCALE)
                        if r >= 0:
                            # token-causal triangle on the leading 128 cols
                            nc.gpsimd.tensor_tensor(
                                p_sb[:, c0 : c0 + 128],
                                p_sb[:, c0 : c0 + 128],
                                cm_sb[:],
                                op=OP.mult,
                            )
                        # gate block mask (broadcast over 64-col blocks) on Pool
                        b0 = j * 8 + (2 * r if r > 0 else 0)
                        msl = mask_sb[:, ti, b0 : (j + 1) * 8]
                        mb = AP(
                            tensor=msl.tensor,
                            offset=msl.offset,
                            ap=list(msl.ap) + [[0, BLK]],
                        )
                        p3 = p_sb[:, cs].rearrange("p (b w) -> p b w", w=BLK)
                        nc.gpsimd.tensor_tensor(p3, p3, mb, op=OP.mult)
                        nc.tensor.matmul(
                            pv_ps[:, cs],
                            v_sb[:, ti, :],
                            p_sb[:, cs],
                            start=(ti == 0),
                            stop=(ti == ntile - 1),
                            skip_group_check=True,
                        )
                        slot += 1
                        if slot * len(ost["queue0"]) // nslots > ost["done"]:
                            ost["done"] += 1
                            emit_oproj_group()
                    rcb = sm.tile([1, 512], BF)
                    with nc.allow_low_precision(reason="recip to bf16 as baseline"):
                        nc.vector.reciprocal(rcb[:], pv_ps[D : D + 1, :])
                    if hh == G - 1:
                        norm_tail(hh, pv_ps, rcb)
                    else:
                        pending = (hh, pv_ps, rcb)

                # hand the completed chunk's o-projection to the filler queue
                while emit_oproj_group():
                    pass
                ost["queue"] = [
                    (si, ej) for si in range(4 * j, 4 * (j + 1)) for ej in range(NE)
                ]
                ost["queue0"] = list(ost["queue"])
                ost["done"] = 0
                ost["attnp"] = attnp
                ost["j"] = j

            # drain the last chunk's o-projection
            while emit_oproj_group(tail=True):
                pass

            if debug:
                for nm, t in [
                    ("dq", q_sb),
                    ("dk", k_sb),
                    ("dv", v_sb),
                    ("dmask", mask_sb),
                    ("dqp", qp_sb),
                    ("dkm", km_sb),
                    ("dkx", kx_sb),
                ]:
                    dd = nc.dram_tensor(
                        nm, list(t[:].shape), t[:].dtype, kind="ExternalOutput"
                    )
                    nc.sync.dma_start(dd[:], t[:])
    return nc


def _host_prep(hidden_states, cos, sin, qkv_w, o_w, gate_wq, gate_wk):
    bf = ml_dtypes.bfloat16
    X = np.asarray(hidden_states, np.float32).reshape(S, HIDDEN)
    qkv_w = np.asarray(qkv_w, np.float32)
    o_w = np.asarray(o_w, np.float32)
    cos = np.asarray(cos, np.float32)
    sin = np.asarray(sin, np.float32)

    xt = np.ascontiguousarray(X.T).astype(bf)
    cosT = np.ascontiguousarray(cos.T).astype(bf)
    sinT = np.ascontiguousarray(sin.T).astype(bf)

    rt = np.zeros((D, D), np.float32)
    h = D // 2
    rt[np.arange(h) + h, np.arange(h)] = -1.0
    rt[np.arange(h), np.arange(h) + h] = 1.0
    rt = rt.astype(bf)

    emat = np.zeros((NB, NT * 128), np.float32)
    for i in range(NT):
        for p in range(128):
            emat[2 * i + p // BLK, i * 128 + p] = 1.0
    eye = np.eye(NB, dtype=np.float32)

    bcm = np.where(
        np.arange(NB)[None, :] <= np.arange(NB)[:, None], 0.0, -60.0
    ).astype(np.float32)
    # row-blocked layouts: [r, j, c] = full[j*8+r, c]
    bcm4 = np.ascontiguousarray(
        (bcm / np.float32(GSCALE)).reshape(NS, 8, NB).transpose(1, 0, 2)
    )
    eye4 = np.ascontiguousarray(
        np.eye(NB, dtype=np.float32).reshape(NS, 8, NB).transpose(1, 0, 2)
    )
    # cm128[p, c] = 1 if c >= p (token causal within a diagonal 128-block)
    p_i = np.arange(128)[:, None]
    c_i = np.arange(128)[None, :]
    cm = (c_i >= p_i).astype(np.float32).astype(bf)

    # k block mean is computed on-device as a SUM; fold 1/BLK into the
    # mean-pool half of gate_wk
    gwk_s = np.asarray(gate_wk, np.float32).copy()
    gwk_s[:D, :] *= 1.0 / BLK

    common = dict(
        xt=xt,
        cosT=cosT,
        sinT=sinT,
        rot=rt,
        gwq=np.asarray(gate_wq, np.float32),
        gwk=gwk_s,
        eye32=eye,
        emat=emat.astype(bf),
        bcm4=bcm4,
        eye4=eye4,
        cm128=cm,
    )
    maps = []
    for c in range(NCORES):
        ow_c = o_w[c * G * D : (c + 1) * G * D, :]  # [384, HIDDEN]
        owp = np.ascontiguousarray(
            ow_c.reshape(NR, 128, HIDDEN).transpose(1, 0, 2)
        ).astype(bf)
        maps.append(
            dict(
                common,
                wq=qkv_w[:, c * G * D : (c + 1) * G * D].astype(bf),
                wk=qkv_w[:, H * D + c * D : H * D + (c + 1) * D].astype(bf),
                wv=qkv_w[
                    :, H * D + HK * D + c * D : H * D + HK * D + (c + 1) * D
                ].astype(bf),
                owp=owp,
            )
        )
    return maps


def _gather(results):
    acc = np.zeros((S, HIDDEN), np.float32)
    for r in results:
        acc += np.asarray(r["out_p"]).astype(np.float32)
    return acc.reshape(1, S, HIDDEN)


def _run(inputs, trace=False):
    global _prog
    if _prog is None:
        _prog = _build()
        if not _prog.is_finalized():
            _prog.finalize()
    from concourse import bass_utils

    maps = _host_prep(**inputs)
    res = bass_utils.run_bass_kernel_spmd(
        _prog, maps, list(range(NCORES)), trace=trace
    )
    return _gather(res.results), res


def kernel(**inputs):
    out, _ = _run(inputs, trace=False)
    return out


# revision 38
# speedup vs baseline: 1.5727x; 1.0523x over previous
import sys

sys.path.insert(0, "/opt/trn_rl_repo")

import numpy as np
import ml_dtypes

# Phi3SeerAttention, B=1 S=2048 HIDDEN=3072, H=32 q heads, HK=8 kv heads,
# D=96, gate block 64, gate hidden 128. Sharded TP over kv heads: core c
# owns kv head c and q heads 4c..4c+3; o-proj row-sharded, partials summed
# on host (the gather step).
H, HK, D, BLK, GH = 32, 8, 96, 64, 128
S, HIDDEN = 2048, 3072
G = H // HK          # 4 q heads per kv head (per core)
NB = S // BLK        # 32 gate blocks
KT = HIDDEN // 128   # 24 contraction tiles
NS = S // 512        # 4 sequence chunks of 512
NT = S // 128        # 16 t-tiles of 128
NE = HIDDEN // 512   # 6 output column chunks
NR = G * D // 128    # 3 packed o-proj contraction tiles
NCORES = 8
THR = 0.03
SCALE = float(D) ** -0.5
GSCALE = (1.0 / (G * BLK)) * float(GH) ** -0.5

_prog = None


def _build(debug=False):
    from concourse import bass, mybir, bacc, library_config
    import concourse.tile as tile
    from concourse.bass import AP

    dt = mybir.dt
    BF, F32 = dt.bfloat16, dt.float32
    AF = mybir.ActivationFunctionType
    OP = mybir.AluOpType
    AX = mybir.AxisListType.X

    nc = bacc.Bacc()
    xt_d = nc.dram_tensor("xt", [HIDDEN, S], BF, kind="ExternalInput")
    wq_d = nc.dram_tensor("wq", [HIDDEN, G * D], BF, kind="ExternalInput")
    wk_d = nc.dram_tensor("wk", [HIDDEN, D], BF, kind="ExternalInput")
    wv_d = nc.dram_tensor("wv", [HIDDEN, D], BF, kind="ExternalInput")
    owp_d = nc.dram_tensor("owp", [128, NR, HIDDEN], BF, kind="ExternalInput")
    cos_d = nc.dram_tensor("cosT", [D, S], BF, kind="ExternalInput")
    sin_d = nc.dram_tensor("sinT", [D, S], BF, kind="ExternalInput")
    rot_d = nc.dram_tensor("rot", [D, D], BF, kind="ExternalInput")
    gwq_d = nc.dram_tensor("gwq", [D, GH], F32, kind="ExternalInput")
    gwk_d = nc.dram_tensor("gwk", [2 * D, GH], F32, kind="ExternalInput")
    eye_d = nc.dram_tensor("eye32", [NB, NB], F32, kind="ExternalInput")
    emat_d = nc.dram_tensor("emat", [NB, NT * 128], BF, kind="ExternalInput")
    bcm4_d = nc.dram_tensor("bcm4", [8, NS, NB], F32, kind="ExternalInput")
    eye4_d = nc.dram_tensor("eye4", [8, NS, NB], F32, kind="ExternalInput")
    cm_d = nc.dram_tensor("cm128", [128, 128], BF, kind="ExternalInput")
    out_d = nc.dram_tensor("out_p", [S, HIDDEN], BF, kind="ExternalOutput")

    # Persistent SBUF tensors (live across the whole pipeline).
    q_sb = nc.alloc_sbuf_tensor("q_sbuf", [D, G, S], BF)
    k_sb = nc.alloc_sbuf_tensor("k_sbuf", [D, S], BF)
    v_sb = nc.alloc_sbuf_tensor("v_sbuf", [128, NT, D + 1], BF)
    qp_sb = nc.alloc_sbuf_tensor("qp_sbuf", [D, G, NB], F32)
    km_sb = nc.alloc_sbuf_tensor("km_sbuf", [D, NB], F32)
    kx_sb = nc.alloc_sbuf_tensor("kx_sbuf", [D, NB], F32)
    mask_sb = nc.alloc_sbuf_tensor("mask_sbuf", [128, NT, NB], BF)

    with tile.TileContext(nc) as tc:
        with tc.tile_pool(name="perm", bufs=1) as perm, tc.tile_pool(
            name="xw", bufs=3
        ) as xw, tc.tile_pool(name="gp", bufs=1) as gp, tc.tile_pool(
            name="ap_", bufs=4
        ) as ap_, tc.tile_pool(name="sm", bufs=4) as sm, tc.tile_pool(
            name="ar", bufs=2
        ) as ar, tc.tile_pool(name="apk", bufs=3) as apk, tc.tile_pool(
            name="ob", bufs=2
        ) as ob, tc.tile_pool(
            name="qps", bufs=2, space="PSUM"
        ) as qps, tc.tile_pool(name="sps", bufs=2, space="PSUM") as sps, tc.tile_pool(
            name="pvs", bufs=2, space="PSUM"
        ) as pvs, tc.tile_pool(name="ops", bufs=2, space="PSUM") as ops:
            # ---- permanent small tensors ----
            wq_sb = perm.tile([128, KT, G * D], BF)
            wk_sb = perm.tile([128, KT, D], BF)
            wv_sb = perm.tile([128, KT, D], BF)
            owp_sb = perm.tile([128, NR, HIDDEN], BF)
            cos_sb = perm.tile([D, S], BF)
            sin_sb = perm.tile([D, S], BF)
            rot_sb = perm.tile([D, D], BF)
            gwq_sb = perm.tile([D, GH], F32)
            gwk_sb = perm.tile([D, 2, GH], F32)
            eye_sb = perm.tile([NB, NB], F32)
            bcm4_sb = perm.tile([8, NS, NB], F32)
            eye4_sb = perm.tile([8, NS, NB], F32)
            emat_sb = perm.tile([NB, NT * 128], BF)
            cm_sb = perm.tile([128, 128], BF)
            ones_sb = perm.tile([1, 128], BF)
            kgT_sb = perm.tile([GH, NB], F32)
            m01t_bf = perm.tile([NB, NB], BF)

            nc.vector.memset(ones_sb[:], 1.0)
            nc.vector.memset(v_sb[:, :, D : D + 1], 1.0)
            nc.vector.memset(kgT_sb[:], 0.0)
            nc.gpsimd.load_library(library_config.standard)

            xt_src = xt_d[:].rearrange("(k p) s -> p k s", p=128)

            # ---- input DMAs, latency-ordered for the serialized DMA device:
            # (wk, xt0) first so PE starts early; cold tables later.
            xt_tiles = []
            for j in range(NS):
                xt_tiles.append(xw.tile([128, KT, 512], BF, name="xt_sb", tag="xt"))
            wk_src = wk_d[:].rearrange("(k p) n -> p k n", p=128)
            nc.sync.dma_start(wk_sb[:, 0:3, :], wk_src[:, 0:3, :])
            nc.sync.dma_start(xt_tiles[0][:, 0:3, :], xt_src[:, 0:3, 0:512])
            nc.sync.dma_start(wk_sb[:, 3:24, :], wk_src[:, 3:24, :])
            nc.sync.dma_start(xt_tiles[0][:, 3:12, :], xt_src[:, 3:12, 0:512])
            nc.sync.dma_start(wv_sb[:], wv_d[:].rearrange("(k p) n -> p k n", p=128))
            nc.sync.dma_start(wq_sb[:], wq_d[:].rearrange("(k p) n -> p k n", p=128))
            nc.sync.dma_start(xt_tiles[0][:, 12:24, :], xt_src[:, 12:24, 0:512])
            nc.sync.dma_start(cos_sb[:], cos_d[:])
            nc.sync.dma_start(sin_sb[:], sin_d[:])
            nc.sync.dma_start(rot_sb[:], rot_d[:])
            nc.sync.dma_start(gwq_sb[:], gwq_d[:])
            nc.sync.dma_start(gwk_sb[:, 0, :], gwk_d[0:D, :])
            nc.sync.dma_start(gwk_sb[:, 1, :], gwk_d[D : 2 * D, :])
            nc.sync.dma_start(eye_sb[:], eye_d[:])
            nc.sync.dma_start(bcm4_sb[:], bcm4_d[:])
            nc.sync.dma_start(eye4_sb[:], eye4_d[:])
            nc.sync.dma_start(emat_sb[:], emat_d[:])
            nc.sync.dma_start(cm_sb[:], cm_d[:])
            nc.sync.dma_start(xt_tiles[1][:], xt_src[:, :, 512:1024])
            nc.sync.dma_start(owp_sb[:], owp_d[:])
            nc.sync.dma_start(xt_tiles[2][:], xt_src[:, :, 1024:1536])
            nc.sync.dma_start(xt_tiles[3][:], xt_src[:, :, 1536:2048])

            # Deferred o-projection: the previous chunk's (si, ej) groups are
            # emitted one per attention tile-slot of the NEXT chunk, so they
            # fill PE while attention is exp(ACT)-paced, and the PE stream
            # never blocks on a whole chunk's attention completing.
            ost = {"queue": [], "osb": None, "n": 0}

            def emit_oproj_group(tail=False):
                if not ost["queue"]:
                    return False
                si, ej, attnp_h, jp = ost["queue"].pop(0)
                if ej == 0:
                    ost["osb"] = ob.tile([128, HIDDEN], BF, name="o_sb")
                o_sb = ost["osb"]
                cc = (si - 4 * jp) * 128
                esl = slice(ej * 512, (ej + 1) * 512)
                if tail:
                    # attention is over: rotate score/projection banks too
                    ost["n"] += 1
                    pool, tg = [(ops, ""), (sps, "s"), (qps, "ps")][ost["n"] % 3]
                    o_ps = pool.tile([128, 512], F32, name="o_ps", tag=tg)
                else:
                    o_ps = ops.tile([128, 512], F32, name="o_ps")
                for rr_ in range(NR):
                    nc.tensor.matmul(
                        o_ps,
                        attnp_h[:, rr_, cc : cc + 128],
                        owp_sb[:, rr_, esl],
                        start=(rr_ == 0),
                        stop=(rr_ == NR - 1),
                    )
                if tail and ej % 2 == 0:
                    nc.scalar.copy(o_sb[:, esl], o_ps[:])
                else:
                    nc.vector.tensor_copy(o_sb[:, esl], o_ps[:])
                if ej == NE - 1:
                    nc.sync.dma_start(out_d[si * 128 : (si + 1) * 128, :], o_sb[:])
                return True

            # ---- fused per-chunk pipeline: projections -> RoPE -> gate ->
            # attention -> deferred o-projection ----
            for j in range(NS):
                sl = slice(j * 512, (j + 1) * 512)
                xt_sb = xt_tiles[j]

                # k chunk
                ps = qps.tile([128, 512], F32)
                pk = ps[:D, :]
                for kt in range(KT):
                    nc.tensor.matmul(
                        pk,
                        wk_sb[:, kt, :],
                        xt_sb[:, kt, :],
                        start=(kt == 0),
                        stop=(kt == KT - 1),
                    )
                pr = pk.rearrange("p (b w) -> p b w", w=BLK)
                bs = slice(j * 8, (j + 1) * 8)
                # block SUM; 1/BLK folded into gwk rows on host
                nc.vector.tensor_reduce(km_sb[:, bs], pr, axis=AX, op=OP.add)
                nc.vector.tensor_reduce(kx_sb[:, bs], pr, axis=AX, op=OP.max)
                nc.scalar.copy(k_sb[:, sl], pk)

                # q heads (before v: the gate chain needs all q pools)
                for hh in range(G):
                    ps = qps.tile([128, 512], F32)
                    pq = ps[:D, :]
                    for kt in range(KT):
                        nc.tensor.matmul(
                            pq,
                            wq_sb[:, kt, hh * D : (hh + 1) * D],
                            xt_sb[:, kt, :],
                            start=(kt == 0),
                            stop=(kt == KT - 1),
                        )
                    pr = pq.rearrange("p (b w) -> p b w", w=BLK)
                    # block SUM; 1/BLK folded into gate scale
                    nc.vector.tensor_reduce(qp_sb[:, hh, bs], pr, axis=AX, op=OP.add)
                    nc.scalar.copy(q_sb[:, hh, sl], pq)

                # RoPE in place for this chunk (k then q heads).
                # rotate_half is a pure partition shift: done with 2 SBUF-to-
                # SBUF DMAs; its sign is folded into the sin table (rows 0:D/2
                # negated on host). Keeps RoPE off PE/PSUM entirely.
                hD = D // 2
                for hh in range(G + 1):
                    src = k_sb[:, sl] if hh == G else q_sb[:, hh, sl]
                    rx = ap_.tile([D, 512], BF, name="rx", bufs=2)
                    nc.sync.dma_start(rx[0:hD, :], src[hD:D, :])
                    nc.sync.dma_start(rx[hD:D, :], src[0:hD, :])
                    t1 = ap_.tile([D, 512], BF, bufs=2)
                    nc.gpsimd.tensor_tensor(t1[:], src, cos_sb[:, sl], op=OP.mult)
                    t2 = ap_.tile([D, 512], BF, bufs=2)
                    nc.vector.tensor_tensor(t2[:], rx[:], sin_sb[:, sl], op=OP.mult)
                    nc.gpsimd.tensor_tensor(src, t1[:], t2[:], op=OP.add)

                # v tiles (transposed layout: s on partitions); PE filler
                # while the serial gate chain runs
                for ti in range(4 * j, 4 * (j + 1)):
                    ps = qps.tile([128, 512], F32)
                    pv = ps[:, :D]
                    for kt in range(KT):
                        nc.tensor.matmul(
                            pv,
                            xt_sb[:, kt, (ti - 4 * j) * 128 : (ti - 4 * j + 1) * 128],
                            wv_sb[:, kt, :],
                            start=(kt == 0),
                            stop=(kt == KT - 1),
                        )
                    nc.scalar.copy(v_sb[:, ti, :D], pv)

                # ---- incremental gate for this chunk's 8 q-blocks ----
                # softmax is row-wise over k-blocks <= q-block, so rows for
                # chunk j only need pools from chunks <= j. Projections are
                # emitted directly transposed (lhsT = gate weight), and the
                # gate scale (1/(G*BLK))*GH^-0.5 is folded into the gate exp
                # (bcm4 is pre-divided by it on host).
                t0g = gp.tile([D, 8], F32)
                qsum = gp.tile([D, 8], F32)
                nc.vector.tensor_add(t0g[:], qp_sb[:, 0, bs], qp_sb[:, 1, bs])
                nc.vector.tensor_add(qsum[:], qp_sb[:, 2, bs], qp_sb[:, 3, bs])
                nc.vector.tensor_add(qsum[:], t0g[:], qsum[:])

                eye8 = eye_sb[0:8, 0:8]
                kgT_ps = pvs.tile([GH, 8], F32, tag="pv", name="kgT_ps")
                nc.tensor.matmul(
                    kgT_ps, gwk_sb[:, 0, :], km_sb[:, bs], start=True, stop=False
                )
                nc.tensor.matmul(
                    kgT_ps, gwk_sb[:, 1, :], kx_sb[:, bs], start=False, stop=True
                )
                qgT_ps = pvs.tile([GH, 8], F32, tag="pv", name="qgT_ps")
                nc.tensor.matmul(qgT_ps, gwq_sb[:], qsum[:], start=True, stop=True)
                qgT_j = gp.tile([GH, 8], F32)
                nc.scalar.copy(qgT_j[:], qgT_ps[:])
                nc.scalar.copy(kgT_sb[:, bs], kgT_ps[:])

                lg_ps = pvs.tile([8, NB], F32, tag="pv", name="lg_ps")
                nc.tensor.matmul(lg_ps, qgT_j[:], kgT_sb[:], start=True, stop=True)
                lm_sb = gp.tile([8, NB], F32)
                nc.vector.tensor_add(lm_sb[:], lg_ps[:], bcm4_sb[:, j, :])
                ge_sb = gp.tile([8, NB], F32)
                gsum = gp.tile([8, 1], F32)
                nc.scalar.activation(
                    ge_sb[:], lm_sb[:], AF.Exp, scale=GSCALE, accum_out=gsum[:]
                )
                grc = gp.tile([8, 1], F32)
                nc.vector.reciprocal(grc[:], gsum[:])
                prob_sb = gp.tile([8, NB], F32)
                nc.scalar.activation(prob_sb[:], ge_sb[:], AF.Copy, scale=grc[:])
                m01 = gp.tile([8, NB], F32)
                nc.vector.tensor_scalar(m01[:], prob_sb[:], THR, None, op0=OP.is_ge)
                nc.vector.tensor_tensor(m01[:], m01[:], eye4_sb[:, j, :], op=OP.max)
                # transpose the 8 new rows into m01t columns
                m01tj_ps = pvs.tile([NB, 8], F32, tag="pv", name="m01tj_ps")
                nc.tensor.matmul(m01tj_ps, m01[:], eye8, start=True, stop=True)
                nc.scalar.copy(m01t_bf[:, bs], m01tj_ps[:])

                # expand k-block rows: mask_sb[p, ti, bs] = m01t[2ti+p//64, bs]
                for ti in range(4 * (j + 1)):
                    mp = sps.tile([128, 512], F32, tag="s", name="mp")
                    mpn = mp[:, :8]
                    nc.tensor.matmul(
                        mpn,
                        emat_sb[:, ti * 128 : (ti + 1) * 128],
                        m01t_bf[:, bs],
                        start=True,
                        stop=True,
                    )
                    nc.scalar.copy(mask_sb[:, ti, bs], mpn)

                # ---- attention for this chunk ----
                # (nslots kept for reference; filler rate is 1 per 2 slots)
                # The normalization tail of head h is emitted inside head
                # h+1's tile loop so the (mostly in-order) PE stream never
                # waits on the DVE reciprocal chain at a head boundary.
                attn_raw = ar.tile([D, G, 512], BF)
                attnp = apk.tile([128, NR, 512], BF)

                def norm_tail(hh, pv_ps, rcb):
                    rb_ps = sps.tile([128, 512], F32, tag="s", name="rb_ps")
                    nc.tensor.matmul(
                        rb_ps[:D, :], ones_sb[:, :D], rcb[:], start=True, stop=True
                    )
                    rb_sb = sm.tile([D, 512], BF, name="rb_sb")
                    nc.vector.tensor_copy(rb_sb[:], rb_ps[:D, :])
                    # DVE: one PSUM input max
                    nc.vector.tensor_tensor(
                        attn_raw[:, hh, :], pv_ps[:D, :], rb_sb[:], op=OP.mult
                    )
                    # repack [96,4,512] -> [128,3,512] as each head lands
                    if hh == 0:
                        nc.sync.dma_start(attnp[0:96, 0, :], attn_raw[:, 0, :])
                    elif hh == 1:
                        nc.sync.dma_start(attnp[96:128, 0, :], attn_raw[0:32, 1, :])
                        nc.sync.dma_start(attnp[0:64, 1, :], attn_raw[32:96, 1, :])
                    elif hh == 2:
                        nc.sync.dma_start(attnp[64:128, 1, :], attn_raw[0:64, 2, :])
                        nc.sync.dma_start(attnp[0:32, 2, :], attn_raw[64:96, 2, :])
                    else:
                        nc.sync.dma_start(attnp[32:128, 2, :], attn_raw[:, 3, :])

                pending = None
                slot = 0
                nslots = G * 4 * (j + 1)
                for hh in range(G):
                    pv_full = pvs.tile([128, 512], F32, tag="pv", name="pv_full")
                    pv_ps = pv_full[: D + 1, :]
                    ntile = 4 * (j + 1)
                    for ti in range(ntile):
                        if ti == 2 and pending is not None:
                            norm_tail(*pending)
                            pending = None
                        r = ti - 4 * j
                        c0 = 128 * r if r > 0 else 0
                        cs = slice(c0, 512)
                        # final chunk: projections are done, so alternate the
                        # score tiles into the idle qps banks (pipeline depth 4)
                        if j == NS - 1 and ti % 2 == 1:
                            s_ps = qps.tile([128, 512], F32, tag="ps", name="s_ps")
                        else:
                            s_ps = sps.tile([128, 512], F32, tag="s")
                        nc.tensor.matmul(
                            s_ps[:, cs],
                            k_sb[:, ti * 128 : (ti + 1) * 128],
                            q_sb[:, hh, j * 512 + c0 : (j + 1) * 512],
                            start=True,
                            stop=True,
                            skip_group_check=True,
                        )
                        p_sb = ap_.tile([128, 512], BF)
                        nc.scalar.activation(p_sb[:, cs], s_ps[:, cs], AF.Exp, scale=SCALE)
                        if r >= 0:
                            # token-causal triangle on the leading 128 cols
                            nc.gpsimd.tensor_tensor(
                                p_sb[:, c0 : c0 + 128],
                                p_sb[:, c0 : c0 + 128],
                                cm_sb[:],
                                op=OP.mult,
                            )
                        # gate block mask (broadcast over 64-col blocks) on Pool
                        b0 = j * 8 + (2 * r if r > 0 else 0)
                        msl = mask_sb[:, ti, b0 : (j + 1) * 8]
                        mb = AP(
                            tensor=msl.tensor,
                            offset=msl.offset,
                            ap=list(msl.ap) + [[0, BLK]],
                        )
                        p3 = p_sb[:, cs].rearrange("p (b w) -> p b w", w=BLK)
                        nc.gpsimd.tensor_tensor(p3, p3, mb, op=OP.mult)
                        nc.tensor.matmul(
                            pv_ps[:, cs],
                            v_sb[:, ti, :],
                            p_sb[:, cs],
                            start=(ti == 0),
                            stop=(ti == ntile - 1),
                            skip_group_check=True,
                        )
                        slot += 1
                        if slot % 2 == 0:
                            emit_oproj_group()
                    rcb = sm.tile([1, 512], BF)
                    with nc.allow_low_precision(reason="recip to bf16 as baseline"):
                        nc.vector.reciprocal(rcb[:], pv_ps[D : D + 1, :])
                    if hh == G - 1:
                        norm_tail(hh, pv_ps, rcb)
                    else:
                        pending = (hh, pv_ps, rcb)

                # hand the completed chunk's o-projection to the filler queue
                ost["queue"] += [
                    (si, ej, attnp, j)
                    for si in range(4 * j, 4 * (j + 1))
                    for ej in range(NE)
                ]

            # drain the last chunk's o-projection
            while emit_oproj_group(tail=True):
                pass

            if debug:
                for nm, t in [
                    ("dq", q_sb),
                    ("dk", k_sb),
                    ("dv", v_sb),
                    ("dmask", mask_sb),
                    ("dqp", qp_sb),
                    ("dkm", km_sb),
                    ("dkx", kx_sb),
                ]:
                    dd = nc.dram_tensor(
                        nm, list(t[:].shape), t[:].dtype, kind="ExternalOutput"
                    )
                    nc.sync.dma_start(dd[:], t[:])
    return nc


def _host_prep(hidden_states, cos, sin, qkv_w, o_w, gate_wq, gate_wk):
    bf = ml_dtypes.bfloat16
    X = np.asarray(hidden_states, np.float32).reshape(S, HIDDEN)
    qkv_w = np.asarray(qkv_w, np.float32)
    o_w = np.asarray(o_w, np.float32)
    cos = np.asarray(cos, np.float32)
    sin = np.asarray(sin, np.float32)

    xt = np.ascontiguousarray(X.T).astype(bf)
    cosT = np.ascontiguousarray(cos.T).astype(bf)
    sinT = np.ascontiguousarray(sin.T).copy()
    sinT[: D // 2, :] *= -1.0
    sinT = sinT.astype(bf)

    rt = np.zeros((D, D), np.float32)
    h = D // 2
    rt[np.arange(h) + h, np.arange(h)] = -1.0
    rt[np.arange(h), np.arange(h) + h] = 1.0
    rt = rt.astype(bf)

    emat = np.zeros((NB, NT * 128), np.float32)
    for i in range(NT):
        for p in range(128):
            emat[2 * i + p // BLK, i * 128 + p] = 1.0
    eye = np.eye(NB, dtype=np.float32)

    bcm = np.where(
        np.arange(NB)[None, :] <= np.arange(NB)[:, None], 0.0, -60.0
    ).astype(np.float32)
    # row-blocked layouts: [r, j, c] = full[j*8+r, c]
    bcm4 = np.ascontiguousarray(
        (bcm / np.float32(GSCALE)).reshape(NS, 8, NB).transpose(1, 0, 2)
    )
    eye4 = np.ascontiguousarray(
        np.eye(NB, dtype=np.float32).reshape(NS, 8, NB).transpose(1, 0, 2)
    )
    # cm128[p, c] = 1 if c >= p (token causal within a diagonal 128-block)
    p_i = np.arange(128)[:, None]
    c_i = np.arange(128)[None, :]
    cm = (c_i >= p_i).astype(np.float32).astype(bf)

    # k block mean is computed on-device as a SUM; fold 1/BLK into the
    # mean-pool half of gate_wk
    gwk_s = np.asarray(gate_wk, np.float32).copy()
    gwk_s[:D, :] *= 1.0 / BLK

    common = dict(
        xt=xt,
        cosT=cosT,
        sinT=sinT,
        rot=rt,
        gwq=np.asarray(gate_wq, np.float32),
        gwk=gwk_s,
        eye32=eye,
        emat=emat.astype(bf),
        bcm4=bcm4,
        eye4=eye4,
        cm128=cm,
    )
    maps = []
    for c in range(NCORES):
        ow_c = o_w[c * G * D : (c + 1) * G * D, :]  # [384, HIDDEN]
        owp = np.ascontiguousarray(
            ow_c.reshape(NR, 128, HIDDEN).transpose(1, 0, 2)
        ).astype(bf)
        maps.append(
            dict(
                common,
                wq=qkv_w[:, c * G * D : (c + 1) * G * D].astype(bf),
                wk=qkv_w[:, H * D + c * D : H * D + (c + 1) * D].astype(bf),
                wv=qkv_w[
                    :, H * D + HK * D + c * D : H * D + HK * D + (c + 1) * D
                ].astype(bf),
                owp=owp,
            )
        )
    return maps


def _gather(results):
    acc = np.zeros((S, HIDDEN), np.float32)
    for r in results:
        acc += np.asarray(r["out_p"]).astype(np.float32)
    return acc.reshape(1, S, HIDDEN)


def _run(inputs, trace=False):
    global _prog
    if _prog is None:
        _prog = _build()
        if not _prog.is_finalized():
            _prog.finalize()
    from concourse import bass_utils

    maps = _host_prep(**inputs)
    res = bass_utils.run_bass_kernel_spmd(
        _prog, maps, list(range(NCORES)), trace=trace
    )
    return _gather(res.results), res


def kernel(**inputs):
    out, _ = _run(inputs, trace=False)
    return out


# revision 41
# speedup vs baseline: 1.6136x; 1.0260x over previous
import sys

sys.path.insert(0, "/opt/trn_rl_repo")

import numpy as np
import ml_dtypes

# Phi3SeerAttention, B=1 S=2048 HIDDEN=3072, H=32 q heads, HK=8 kv heads,
# D=96, gate block 64, gate hidden 128. Sharded TP over kv heads: core c
# owns kv head c and q heads 4c..4c+3; o-proj row-sharded, partials summed
# on host (the gather step).
H, HK, D, BLK, GH = 32, 8, 96, 64, 128
S, HIDDEN = 2048, 3072
G = H // HK          # 4 q heads per kv head (per core)
NB = S // BLK        # 32 gate blocks
KT = HIDDEN // 128   # 24 contraction tiles
NS = S // 512        # 4 sequence chunks of 512
NT = S // 128        # 16 t-tiles of 128
NE = HIDDEN // 512   # 6 output column chunks
NR = G * D // 128    # 3 packed o-proj contraction tiles
NCORES = 8
THR = 0.03
SCALE = float(D) ** -0.5
GSCALE = (1.0 / (G * BLK)) * float(GH) ** -0.5

_prog = None


def _build(debug=False):
    from concourse import bass, mybir, bacc, library_config
    import concourse.tile as tile
    from concourse.bass import AP

    dt = mybir.dt
    BF, F32 = dt.bfloat16, dt.float32
    AF = mybir.ActivationFunctionType
    OP = mybir.AluOpType
    AX = mybir.AxisListType.X

    nc = bacc.Bacc()
    xt_d = nc.dram_tensor("xt", [HIDDEN, S], BF, kind="ExternalInput")
    wq_d = nc.dram_tensor("wq", [HIDDEN, G * D], BF, kind="ExternalInput")
    wk_d = nc.dram_tensor("wk", [HIDDEN, D], BF, kind="ExternalInput")
    wv_d = nc.dram_tensor("wv", [HIDDEN, D], BF, kind="ExternalInput")
    owp_d = nc.dram_tensor("owp", [128, NR, HIDDEN], BF, kind="ExternalInput")
    cos_d = nc.dram_tensor("cosT", [D, S], BF, kind="ExternalInput")
    sin_d = nc.dram_tensor("sinT", [D, S], BF, kind="ExternalInput")
    gwq_d = nc.dram_tensor("gwq", [D, GH], F32, kind="ExternalInput")
    gwk_d = nc.dram_tensor("gwk", [2 * D, GH], F32, kind="ExternalInput")
    eye_d = nc.dram_tensor("eye32", [NB, NB], F32, kind="ExternalInput")
    emat_d = nc.dram_tensor("emat", [NB, NT * 128], BF, kind="ExternalInput")
    bcm4_d = nc.dram_tensor("bcm4", [8, NS, NB], F32, kind="ExternalInput")
    eye4_d = nc.dram_tensor("eye4", [8, NS, NB], F32, kind="ExternalInput")
    cm_d = nc.dram_tensor("cm128", [128, 128], BF, kind="ExternalInput")
    out_d = nc.dram_tensor("out_p", [S, HIDDEN], BF, kind="ExternalOutput")

    # Persistent SBUF tensors (live across the whole pipeline).
    q_sb = nc.alloc_sbuf_tensor("q_sbuf", [D, G, S], BF)
    k_sb = nc.alloc_sbuf_tensor("k_sbuf", [D, S], BF)
    v_sb = nc.alloc_sbuf_tensor("v_sbuf", [128, NT, D + 1], BF)
    qp_sb = nc.alloc_sbuf_tensor("qp_sbuf", [D, G, NB], F32)
    km_sb = nc.alloc_sbuf_tensor("km_sbuf", [D, NB], F32)
    kx_sb = nc.alloc_sbuf_tensor("kx_sbuf", [D, NB], F32)
    mask_sb = nc.alloc_sbuf_tensor("mask_sbuf", [128, NT, NB], BF)

    with tile.TileContext(nc) as tc:
        with tc.tile_pool(name="perm", bufs=1) as perm, tc.tile_pool(
            name="xw", bufs=3
        ) as xw, tc.tile_pool(name="gp", bufs=1) as gp, tc.tile_pool(
            name="ap_", bufs=4
        ) as ap_, tc.tile_pool(name="sm", bufs=4) as sm, tc.tile_pool(
            name="ar", bufs=2
        ) as ar, tc.tile_pool(name="apk", bufs=3) as apk, tc.tile_pool(
            name="ob", bufs=2
        ) as ob, tc.tile_pool(
            name="qps", bufs=2, space="PSUM"
        ) as qps, tc.tile_pool(name="sps", bufs=2, space="PSUM") as sps, tc.tile_pool(
            name="pvs", bufs=2, space="PSUM"
        ) as pvs, tc.tile_pool(name="ops", bufs=2, space="PSUM") as ops:
            # ---- permanent small tensors ----
            wq_sb = perm.tile([128, KT, G * D], BF)
            wk_sb = perm.tile([128, KT, D], BF)
            wv_sb = perm.tile([128, KT, D], BF)
            owp_sb = perm.tile([128, NR, HIDDEN], BF)
            cos_sb = perm.tile([D, S], BF)
            sin_sb = perm.tile([D, S], BF)
            gwq_sb = perm.tile([D, GH], F32)
            gwk_sb = perm.tile([D, 2, GH], F32)
            eye_sb = perm.tile([NB, NB], F32)
            bcm4_sb = perm.tile([8, NS, NB], F32)
            eye4_sb = perm.tile([8, NS, NB], F32)
            emat_sb = perm.tile([NB, NT * 128], BF)
            cm_sb = perm.tile([128, 128], BF)
            ones_sb = perm.tile([1, 128], BF)
            kgT_sb = perm.tile([GH, NB], F32)
            m01t_bf = perm.tile([NB, NB], BF)

            nc.vector.memset(ones_sb[:], 1.0)
            nc.vector.memset(v_sb[:, :, D : D + 1], 1.0)
            nc.vector.memset(kgT_sb[:], 0.0)
            nc.gpsimd.load_library(library_config.standard)

            xt_src = xt_d[:].rearrange("(k p) s -> p k s", p=128)

            # ---- input DMAs, latency-ordered for the serialized DMA device:
            # (wk, xt0) first so PE starts early; cold tables later.
            xt_tiles = []
            for j in range(NS):
                xt_tiles.append(xw.tile([128, KT, 512], BF, name="xt_sb", tag="xt"))
            wk_src = wk_d[:].rearrange("(k p) n -> p k n", p=128)
            nc.sync.dma_start(wk_sb[:, 0:3, :], wk_src[:, 0:3, :])
            nc.sync.dma_start(xt_tiles[0][:, 0:3, 0:256], xt_src[:, 0:3, 0:256])
            nc.sync.dma_start(wk_sb[:, 3:24, :], wk_src[:, 3:24, :])
            nc.sync.dma_start(xt_tiles[0][:, 3:24, 0:256], xt_src[:, 3:24, 0:256])
            nc.sync.dma_start(wq_sb[:], wq_d[:].rearrange("(k p) n -> p k n", p=128))
            nc.sync.dma_start(xt_tiles[0][:, :, 256:512], xt_src[:, :, 256:512])
            nc.sync.dma_start(wv_sb[:], wv_d[:].rearrange("(k p) n -> p k n", p=128))
            nc.sync.dma_start(cos_sb[:], cos_d[:])
            nc.sync.dma_start(sin_sb[:], sin_d[:])
            nc.sync.dma_start(gwq_sb[:], gwq_d[:])
            nc.sync.dma_start(gwk_sb[:, 0, :], gwk_d[0:D, :])
            nc.sync.dma_start(gwk_sb[:, 1, :], gwk_d[D : 2 * D, :])
            nc.sync.dma_start(eye_sb[:], eye_d[:])
            nc.sync.dma_start(bcm4_sb[:], bcm4_d[:])
            nc.sync.dma_start(eye4_sb[:], eye4_d[:])
            nc.sync.dma_start(emat_sb[:], emat_d[:])
            nc.sync.dma_start(cm_sb[:], cm_d[:])
            nc.sync.dma_start(xt_tiles[1][:], xt_src[:, :, 512:1024])
            nc.sync.dma_start(owp_sb[:], owp_d[:])
            nc.sync.dma_start(xt_tiles[2][:], xt_src[:, :, 1024:1536])
            nc.sync.dma_start(xt_tiles[3][:], xt_src[:, :, 1536:2048])

            # Deferred o-projection: the previous chunk's (si, ej) groups are
            # emitted one per attention tile-slot of the NEXT chunk, so they
            # fill PE while attention is exp(ACT)-paced, and the PE stream
            # never blocks on a whole chunk's attention completing.
            ost = {"queue": [], "osb": None, "n": 0}

            def emit_oproj_group(tail=False):
                if not ost["queue"]:
                    return False
                si, ej, attnp_h, jp = ost["queue"].pop(0)
                if ej == 0:
                    ost["osb"] = ob.tile([128, HIDDEN], BF, name="o_sb")
                o_sb = ost["osb"]
                cc = (si - 4 * jp) * 128
                esl = slice(ej * 512, (ej + 1) * 512)
                if tail:
                    # attention is over: rotate score/projection banks too
                    ost["n"] += 1
                    pool, tg = [(ops, ""), (sps, "s"), (qps, "ps")][ost["n"] % 3]
                    o_ps = pool.tile([128, 512], F32, name="o_ps", tag=tg)
                else:
                    o_ps = ops.tile([128, 512], F32, name="o_ps")
                for rr_ in range(NR):
                    nc.tensor.matmul(
                        o_ps,
                        attnp_h[:, rr_, cc : cc + 128],
                        owp_sb[:, rr_, esl],
                        start=(rr_ == 0),
                        stop=(rr_ == NR - 1),
                    )
                if tail and ej % 2 == 0:
                    nc.scalar.copy(o_sb[:, esl], o_ps[:])
                else:
                    nc.vector.tensor_copy(o_sb[:, esl], o_ps[:])
                if ej == NE - 1:
                    nc.sync.dma_start(out_d[si * 128 : (si + 1) * 128, :], o_sb[:])
                return True

            # ---- fused per-chunk pipeline: projections -> RoPE -> gate ->
            # attention -> deferred o-projection ----
            for j in range(NS):
                sl = slice(j * 512, (j + 1) * 512)
                xt_sb = xt_tiles[j]
                bs = slice(j * 8, (j + 1) * 8)

                # chunk 0 is DMA-latency-critical: run its k/q projections in
                # column halves so PE starts once half of xt0 has landed
                halves = [(0, 256), (256, 256)] if j == 0 else [(0, 512)]
                for c0w, w in halves:
                    ccol = slice(c0w, c0w + w)
                    sl2 = slice(j * 512 + c0w, j * 512 + c0w + w)
                    bs2 = slice(j * 8 + c0w // 64, j * 8 + (c0w + w) // 64)

                    # k
                    ps = qps.tile([128, 512], F32)
                    pk = ps[:D, :w]
                    for kt in range(KT):
                        nc.tensor.matmul(
                            pk,
                            wk_sb[:, kt, :],
                            xt_sb[:, kt, ccol],
                            start=(kt == 0),
                            stop=(kt == KT - 1),
                        )
                    pr = pk.rearrange("p (b w) -> p b w", w=BLK)
                    # block SUM; 1/BLK folded into gwk rows on host
                    nc.vector.tensor_reduce(km_sb[:, bs2], pr, axis=AX, op=OP.add)
                    nc.vector.tensor_reduce(kx_sb[:, bs2], pr, axis=AX, op=OP.max)
                    nc.scalar.copy(k_sb[:, sl2], pk)

                    # q heads (before v: the gate chain needs all q pools)
                    for hh in range(G):
                        ps = qps.tile([128, 512], F32)
                        pq = ps[:D, :w]
                        for kt in range(KT):
                            nc.tensor.matmul(
                                pq,
                                wq_sb[:, kt, hh * D : (hh + 1) * D],
                                xt_sb[:, kt, ccol],
                                start=(kt == 0),
                                stop=(kt == KT - 1),
                            )
                        pr = pq.rearrange("p (b w) -> p b w", w=BLK)
                        # block SUM; 1/BLK folded into gate scale
                        nc.vector.tensor_reduce(
                            qp_sb[:, hh, bs2], pr, axis=AX, op=OP.add
                        )
                        nc.scalar.copy(q_sb[:, hh, sl2], pq)

                # RoPE in place for this chunk (k then q heads).
                # rotate_half is a pure partition shift: done with 2 SBUF-to-
                # SBUF DMAs; its sign is folded into the sin table (rows 0:D/2
                # negated on host). Keeps RoPE off PE/PSUM entirely.
                hD = D // 2
                for hh in range(G + 1):
                    src = k_sb[:, sl] if hh == G else q_sb[:, hh, sl]
                    rx = ap_.tile([D, 512], BF, name="rx", bufs=2)
                    nc.sync.dma_start(rx[0:hD, :], src[hD:D, :])
                    nc.sync.dma_start(rx[hD:D, :], src[0:hD, :])
                    t1 = ap_.tile([D, 512], BF, bufs=2)
                    nc.gpsimd.tensor_tensor(t1[:], src, cos_sb[:, sl], op=OP.mult)
                    t2 = ap_.tile([D, 512], BF, bufs=2)
                    nc.vector.tensor_tensor(t2[:], rx[:], sin_sb[:, sl], op=OP.mult)
                    nc.gpsimd.tensor_tensor(src, t1[:], t2[:], op=OP.add)

                # v tiles (transposed layout: s on partitions); PE filler
                # while the serial gate chain runs
                for ti in range(4 * j, 4 * (j + 1)):
                    ps = qps.tile([128, 512], F32)
                    pv = ps[:, :D]
                    for kt in range(KT):
                        nc.tensor.matmul(
                            pv,
                            xt_sb[:, kt, (ti - 4 * j) * 128 : (ti - 4 * j + 1) * 128],
                            wv_sb[:, kt, :],
                            start=(kt == 0),
                            stop=(kt == KT - 1),
                        )
                    nc.scalar.copy(v_sb[:, ti, :D], pv)

                # ---- incremental gate for this chunk's 8 q-blocks ----
                # softmax is row-wise over k-blocks <= q-block, so rows for
                # chunk j only need pools from chunks <= j. Projections are
                # emitted directly transposed (lhsT = gate weight), and the
                # gate scale (1/(G*BLK))*GH^-0.5 is folded into the gate exp
                # (bcm4 is pre-divided by it on host).
                t0g = gp.tile([D, 8], F32)
                qsum = gp.tile([D, 8], F32)
                nc.vector.tensor_add(t0g[:], qp_sb[:, 0, bs], qp_sb[:, 1, bs])
                nc.vector.tensor_add(qsum[:], qp_sb[:, 2, bs], qp_sb[:, 3, bs])
                nc.vector.tensor_add(qsum[:], t0g[:], qsum[:])

                eye8 = eye_sb[0:8, 0:8]
                kgT_ps = pvs.tile([GH, 8], F32, tag="pv", name="kgT_ps")
                nc.tensor.matmul(
                    kgT_ps, gwk_sb[:, 0, :], km_sb[:, bs], start=True, stop=False
                )
                nc.tensor.matmul(
                    kgT_ps, gwk_sb[:, 1, :], kx_sb[:, bs], start=False, stop=True
                )
                qgT_ps = pvs.tile([GH, 8], F32, tag="pv", name="qgT_ps")
                nc.tensor.matmul(qgT_ps, gwq_sb[:], qsum[:], start=True, stop=True)
                qgT_j = gp.tile([GH, 8], F32)
                nc.scalar.copy(qgT_j[:], qgT_ps[:])
                nc.scalar.copy(kgT_sb[:, bs], kgT_ps[:])

                lg_ps = pvs.tile([8, NB], F32, tag="pv", name="lg_ps")
                nc.tensor.matmul(lg_ps, qgT_j[:], kgT_sb[:], start=True, stop=True)
                lm_sb = gp.tile([8, NB], F32)
                nc.vector.tensor_add(lm_sb[:], lg_ps[:], bcm4_sb[:, j, :])
                ge_sb = gp.tile([8, NB], F32)
                gsum = gp.tile([8, 1], F32)
                nc.scalar.activation(
                    ge_sb[:], lm_sb[:], AF.Exp, scale=GSCALE, accum_out=gsum[:]
                )
                grc = gp.tile([8, 1], F32)
                nc.vector.reciprocal(grc[:], gsum[:])
                prob_sb = gp.tile([8, NB], F32)
                nc.scalar.activation(prob_sb[:], ge_sb[:], AF.Copy, scale=grc[:])
                m01 = gp.tile([8, NB], F32)
                nc.vector.tensor_scalar(m01[:], prob_sb[:], THR, None, op0=OP.is_ge)
                nc.vector.tensor_tensor(m01[:], m01[:], eye4_sb[:, j, :], op=OP.max)
                # transpose the 8 new rows into m01t columns
                m01tj_ps = pvs.tile([NB, 8], F32, tag="pv", name="m01tj_ps")
                nc.tensor.matmul(m01tj_ps, m01[:], eye8, start=True, stop=True)
                nc.scalar.copy(m01t_bf[:, bs], m01tj_ps[:])

                # expand k-block rows: mask_sb[p, ti, bs] = m01t[2ti+p//64, bs]
                for ti in range(4 * (j + 1)):
                    mp = sps.tile([128, 512], F32, tag="s", name="mp")
                    mpn = mp[:, :8]
                    nc.tensor.matmul(
                        mpn,
                        emat_sb[:, ti * 128 : (ti + 1) * 128],
                        m01t_bf[:, bs],
                        start=True,
                        stop=True,
                    )
                    nc.scalar.copy(mask_sb[:, ti, bs], mpn)

                # ---- attention for this chunk ----
                # (nslots kept for reference; filler rate is 1 per 2 slots)
                # The normalization tail of head h is emitted inside head
                # h+1's tile loop so the (mostly in-order) PE stream never
                # waits on the DVE reciprocal chain at a head boundary.
                attn_raw = ar.tile([D, G, 512], BF)
                attnp = apk.tile([128, NR, 512], BF)

                def norm_tail(hh, pv_ps, rcb):
                    rb_ps = sps.tile([128, 512], F32, tag="s", name="rb_ps")
                    nc.tensor.matmul(
                        rb_ps[:D, :], ones_sb[:, :D], rcb[:], start=True, stop=True
                    )
                    rb_sb = sm.tile([D, 512], BF, name="rb_sb")
                    nc.vector.tensor_copy(rb_sb[:], rb_ps[:D, :])
                    # DVE: one PSUM input max
                    nc.vector.tensor_tensor(
                        attn_raw[:, hh, :], pv_ps[:D, :], rb_sb[:], op=OP.mult
                    )
                    # repack [96,4,512] -> [128,3,512] as each head lands
                    if hh == 0:
                        nc.sync.dma_start(attnp[0:96, 0, :], attn_raw[:, 0, :])
                    elif hh == 1:
                        nc.sync.dma_start(attnp[96:128, 0, :], attn_raw[0:32, 1, :])
                        nc.sync.dma_start(attnp[0:64, 1, :], attn_raw[32:96, 1, :])
                    elif hh == 2:
                        nc.sync.dma_start(attnp[64:128, 1, :], attn_raw[0:64, 2, :])
                        nc.sync.dma_start(attnp[0:32, 2, :], attn_raw[64:96, 2, :])
                    else:
                        nc.sync.dma_start(attnp[32:128, 2, :], attn_raw[:, 3, :])

                pending = None
                slot = 0
                nslots = G * 4 * (j + 1)
                for hh in range(G):
                    pv_full = pvs.tile([128, 512], F32, tag="pv", name="pv_full")
                    pv_ps = pv_full[: D + 1, :]
                    ntile = 4 * (j + 1)
                    for ti in range(ntile):
                        if ti == 2 and pending is not None:
                            norm_tail(*pending)
                            pending = None
                        r = ti - 4 * j
                        c0 = 128 * r if r > 0 else 0
                        cs = slice(c0, 512)
                        # final chunk: projections are done, so alternate the
                        # score tiles into the idle qps banks (pipeline depth 4)
                        if j == NS - 1 and ti % 2 == 1:
                            s_ps = qps.tile([128, 512], F32, tag="ps", name="s_ps")
                        else:
                            s_ps = sps.tile([128, 512], F32, tag="s")
                        nc.tensor.matmul(
                            s_ps[:, cs],
                            k_sb[:, ti * 128 : (ti + 1) * 128],
                            q_sb[:, hh, j * 512 + c0 : (j + 1) * 512],
                            start=True,
                            stop=True,
                            skip_group_check=True,
                        )
                        p_sb = ap_.tile([128, 512], BF)
                        nc.scalar.activation(p_sb[:, cs], s_ps[:, cs], AF.Exp, scale=SCALE)
                        if r >= 0:
                            # token-causal triangle on the leading 128 cols
                            nc.gpsimd.tensor_tensor(
                                p_sb[:, c0 : c0 + 128],
                                p_sb[:, c0 : c0 + 128],
                                cm_sb[:],
                                op=OP.mult,
                            )
                        # gate block mask (broadcast over 64-col blocks) on Pool
                        b0 = j * 8 + (2 * r if r > 0 else 0)
                        msl = mask_sb[:, ti, b0 : (j + 1) * 8]
                        mb = AP(
                            tensor=msl.tensor,
                            offset=msl.offset,
                            ap=list(msl.ap) + [[0, BLK]],
                        )
                        p3 = p_sb[:, cs].rearrange("p (b w) -> p b w", w=BLK)
                        nc.gpsimd.tensor_tensor(p3, p3, mb, op=OP.mult)
                        nc.tensor.matmul(
                            pv_ps[:, cs],
                            v_sb[:, ti, :],
                            p_sb[:, cs],
                            start=(ti == 0),
                            stop=(ti == ntile - 1),
                            skip_group_check=True,
                        )
                        slot += 1
                        if slot % 2 == 0:
                            emit_oproj_group()
                    rcb = sm.tile([1, 512], BF)
                    with nc.allow_low_precision(reason="recip to bf16 as baseline"):
                        nc.vector.reciprocal(rcb[:], pv_ps[D : D + 1, :])
                    if hh == G - 1:
                        norm_tail(hh, pv_ps, rcb)
                    else:
                        pending = (hh, pv_ps, rcb)

                # hand the completed chunk's o-projection to the filler queue
                ost["queue"] += [
                    (si, ej, attnp, j)
                    for si in range(4 * j, 4 * (j + 1))
                    for ej in range(NE)
                ]

            # drain the last chunk's o-projection
            while emit_oproj_group(tail=True):
                pass

            if debug:
                for nm, t in [
                    ("dq", q_sb),
                    ("dk", k_sb),
                    ("dv", v_sb),
                    ("dmask", mask_sb),
                    ("dqp", qp_sb),
                    ("dkm", km_sb),
                    ("dkx", kx_sb),
                ]:
                    dd = nc.dram_tensor(
                        nm, list(t[:].shape), t[:].dtype, kind="ExternalOutput"
                    )
                    nc.sync.dma_start(dd[:], t[:])
    return nc


def _host_prep(hidden_states, cos, sin, qkv_w, o_w, gate_wq, gate_wk):
    bf = ml_dtypes.bfloat16
    X = np.asarray(hidden_states, np.float32).reshape(S, HIDDEN)
    qkv_w = np.asarray(qkv_w, np.float32)
    o_w = np.asarray(o_w, np.float32)
    cos = np.asarray(cos, np.float32)
    sin = np.asarray(sin, np.float32)

    xt = np.ascontiguousarray(X.T).astype(bf)
    cosT = np.ascontiguousarray(cos.T).astype(bf)
    sinT = np.ascontiguousarray(sin.T).copy()
    sinT[: D // 2, :] *= -1.0
    sinT = sinT.astype(bf)

    emat = np.zeros((NB, NT * 128), np.float32)
    for i in range(NT):
        for p in range(128):
            emat[2 * i + p // BLK, i * 128 + p] = 1.0
    eye = np.eye(NB, dtype=np.float32)

    bcm = np.where(
        np.arange(NB)[None, :] <= np.arange(NB)[:, None], 0.0, -60.0
    ).astype(np.float32)
    # row-blocked layouts: [r, j, c] = full[j*8+r, c]
    bcm4 = np.ascontiguousarray(
        (bcm / np.float32(GSCALE)).reshape(NS, 8, NB).transpose(1, 0, 2)
    )
    eye4 = np.ascontiguousarray(
        np.eye(NB, dtype=np.float32).reshape(NS, 8, NB).transpose(1, 0, 2)
    )
    # cm128[p, c] = 1 if c >= p (token causal within a diagonal 128-block)
    p_i = np.arange(128)[:, None]
    c_i = np.arange(128)[None, :]
    cm = (c_i >= p_i).astype(np.float32).astype(bf)

    # k block mean is computed on-device as a SUM; fold 1/BLK into the
    # mean-pool half of gate_wk
    gwk_s = np.asarray(gate_wk, np.float32).copy()
    gwk_s[:D, :] *= 1.0 / BLK

    common = dict(
        xt=xt,
        cosT=cosT,
        sinT=sinT,
        gwq=np.asarray(gate_wq, np.float32),
        gwk=gwk_s,
        eye32=eye,
        emat=emat.astype(bf),
        bcm4=bcm4,
        eye4=eye4,
        cm128=cm,
    )
    maps = []
    for c in range(NCORES):
        ow_c = o_w[c * G * D : (c + 1) * G * D, :]  # [384, HIDDEN]
        owp = np.ascontiguousarray(
            ow_c.reshape(NR, 128, HIDDEN).transpose(1, 0, 2)
        ).astype(bf)
        maps.append(
            dict(
                common,
                wq=qkv_w[:, c * G * D : (c + 1) * G * D].astype(bf),
                wk=qkv_w[:, H * D + c * D : H * D + (c + 1) * D].astype(bf),
                wv=qkv_w[
                    :, H * D + HK * D + c * D : H * D + HK * D + (c + 1) * D
                ].astype(bf),
                owp=owp,
            )
        )
    return maps


def _gather(results):
    acc = np.zeros((S, HIDDEN), np.float32)
    for r in results:
        acc += np.asarray(r["out_p"]).astype(np.float32)
    return acc.reshape(1, S, HIDDEN)


def _run(inputs, trace=False):
    global _prog
    if _prog is None:
        _prog = _build()
        if not _prog.is_finalized():
            _prog.finalize()
    from concourse import bass_utils

    maps = _host_prep(**inputs)
    res = bass_utils.run_bass_kernel_spmd(
        _prog, maps, list(range(NCORES)), trace=trace
    )
    return _gather(res.results), res


def kernel(**inputs):
    out, _ = _run(inputs, trace=False)
    return out
